# revision 1
# baseline (speedup 1.0000x reference)
"""Bass/Trainium2 kernel for 2-layer GCN (nn_MeshGNN), 8 NeuronCores.

Math (per layer, commuted form):
    A_hat = D^-1/2 (A+I) D^-1/2 ;  gcn(x) = A_hat x W + b
    u = dinv * x ;  agg[d] = sum_{e: dst=d} u[src[e]] + u[d]
    out = (dinv * agg) @ W + b           (layer 1 adds relu)

Distribution: nodes sharded by range across 8 cores (12500/core, padded to
12544). Each core aggregates its own dst range; the gather table u (fp16,
[100352, 128] rows = 256B) is replicated via AllGather between layers.
Edge streams (gather indices + relative-dst) are identical for both layers.
"""
import os
import numpy as np

import concourse.bacc as bacc
import concourse.mybir as mybir
from concourse.tile import TileContext
from concourse.bass_utils import run_bass_kernel_spmd

# ---------------------------------------------------------------- constants
N_NODES = 100000
NC_CORES = 8
S = 12500                 # nodes per core
TS = 128                  # dst-tile size
TPC = 98                  # dst tiles per core (98*128 = 12544)
SP = TPC * TS             # padded nodes per core
NCH = 4                   # src chunks (int16 gather index limit)
CH = SP * NC_CORES // NCH # 25088 chunk rows
FD = 64                   # in/hidden feature dim
OD = 32                   # output dim
BLK = int(os.environ.get("KGNN_BLK", "2048"))   # gather block (slots per dma_gather)
OHG = 1024                # one-hot group (slots per DVE op)
_NO_GATHER = bool(int(os.environ.get("KGNN_NO_GATHER", "0")))
# single_packet=True coalesces all of a gather's descriptors into one DMA
# packet; beyond ~64 descriptors/lane that exceeds the packet ceiling and
# wedges the SDMA engines, so large gathers must use False.
_SINGLE_PACKET = bool(int(os.environ.get("KGNN_SINGLE_PACKET", "0")))
F32 = mybir.dt.float32
F16 = mybir.dt.float16
I16 = mybir.dt.int16

_compiled_cache = {}


# ---------------------------------------------------------------- tile patch
def _install_tile_patch():
    """walrus here rejects >1 sync-wait on an InstDrain; split the Tile tail
    drain's waits across sequential drains (same engine => same semantics)."""
    from bass_rust import ScopedClock

    def _patched(self, tick_clock, wait_clock):
        drain_inst = self.nc.sync.drain()
        wait_clock.add_sem_waits(
            drain_inst.ins, ScopedClock({None: tick_clock.global_clock})
        )
        si = drain_inst.ins.sync_info
        waits = list(si.on_wait) if si and si.on_wait else []
        if len(waits) > 1:
            si.on_wait = waits[:1]
            for w in waits[1:]:
                extra = self.nc.sync.drain()
                extra.ins.sync_info = mybir.SyncInfo(on_wait=[w], on_update=[])
        self.nc.all_engine_barrier()
        assert self.sems is not None
        popped = self.nc._tile_sem_poison_stack.pop()
        assert popped is self._sem_poison
        self.nc.clear_and_free_semaphores(list(self.sems.allocated().values()))
        self.nc.all_engine_barrier()

    TileContext._drain_and_barrier = _patched


_install_tile_patch()


# ---------------------------------------------------------------- host prep
def _prep_edges(edge_index):
    """Shared-run-structure edge streams. Returns (runs_padded [NCH,TPC],
    idx_streams [NC][tot], rel_streams [NC][tot], sec_base [NCH], tot)."""
    src = np.asarray(edge_index[0], dtype=np.int64)
    dst = np.asarray(edge_index[1], dtype=np.int64)
    src_row = (src // S) * SP + (src % S)          # padded table row
    chunk = src_row // CH
    core = dst // S
    dstloc = dst - core * S
    tile = dstloc // TS

    key = (core * NCH + chunk) * TPC + tile
    counts = np.bincount(key, minlength=NC_CORES * NCH * TPC).reshape(
        NC_CORES, NCH, TPC
    )
    runs = counts.max(axis=0)                       # [NCH, TPC]
    runs_padded = ((runs + TS - 1) // TS) * TS      # mult of 128
    sec_len = runs_padded.sum(axis=1)               # [NCH]
    sec_base = np.concatenate([[0], np.cumsum(sec_len)[:-1]])
    run_start = sec_base[:, None] + (
        np.cumsum(runs_padded, axis=1) - runs_padded
    )                                               # [NCH, TPC] global start
    tot = int(sec_len.sum())

    idx_streams, rel_streams = [], []
    for k in range(NC_CORES):
        sel = core == k
        c_k, t_k = chunk[sel], tile[sel]
        row_k = src_row[sel] % CH
        rel_k = (dstloc[sel] - t_k * TS).astype(np.float16)
        order = np.lexsort((t_k, c_k))
        c_k, t_k, row_k, rel_k = c_k[order], t_k[order], row_k[order], rel_k[order]
        # position within each (c, t) run
        key_k = c_k * TPC + t_k
        cnt_k = np.bincount(key_k, minlength=NCH * TPC)
        grp_start = np.cumsum(cnt_k) - cnt_k
        within = np.arange(len(key_k)) - grp_start[key_k]
        slot = run_start.reshape(-1)[key_k] + within
        idx_s = np.zeros(tot, dtype=np.int16)
        rel_s = np.full(tot, -1.0, dtype=np.float16)
        idx_s[slot] = row_k.astype(np.int16)
        rel_s[slot] = rel_k
        idx_streams.append(idx_s)
        rel_streams.append(rel_s)
    return runs_padded, run_start, sec_base, sec_len, tot, idx_streams, rel_streams


def _wrap_idx(idx_s):
    """[tot] int16 -> [128, tot/16] wrapped + replicated across 8 groups."""
    tot = idx_s.shape[0]
    w = idx_s.reshape(tot // 16, 16).T              # [16, tot/16]
    return np.tile(w, (8, 1)).copy()                # [128, tot/16]


def _wrap_rel(rel_s):
    tot = rel_s.shape[0]
    return rel_s.reshape(tot // 128, 128).T.copy()  # [128, tot/128]


# ---------------------------------------------------------------- kernel build
def _build(runs_padded, run_start, sec_base, sec_len, tot):
    nc = bacc.Bacc(None, target_bir_lowering=False, debug=False,
                   num_devices=NC_CORES, num_swdge_queues=4)

    # ---- I/O -------------------------------------------------------------
    d_x = nc.dram_tensor("x_shard", [128, TPC, FD], F32, kind="ExternalInput")
    d_deg = nc.dram_tensor("deg_shard", [128, TPC], F32, kind="ExternalInput")
    d_idx = nc.dram_tensor("idx_stream", [128, tot // 16], I16, kind="ExternalInput")
    d_rel = nc.dram_tensor("rel_stream", [128, tot // 128], F16, kind="ExternalInput")
    d_iota = nc.dram_tensor("iota16", [128, TS], F16, kind="ExternalInput")
    d_id32 = nc.dram_tensor("ident32", [128, 128], F32, kind="ExternalInput")
    d_id16 = nc.dram_tensor("ident16", [128, 128], F16, kind="ExternalInput")
    d_w1 = nc.dram_tensor("W1", [FD, FD], F32, kind="ExternalInput")
    d_b1 = nc.dram_tensor("b1rep", [128, FD], F32, kind="ExternalInput")
    d_w2 = nc.dram_tensor("W2", [FD, OD], F32, kind="ExternalInput")
    d_b2 = nc.dram_tensor("b2rep", [128, OD], F32, kind="ExternalInput")
    d_out = nc.dram_tensor("out_shard", [128, TPC, OD], F32, kind="ExternalOutput")

    cc_in1 = nc.dram_tensor("cc_in1", [SP, 128], F16, kind="Internal")
    u1full = nc.dram_tensor("u1full", [SP * NC_CORES, 128], F16,
                            kind="Internal", addr_space="Shared")
    cc_in2 = nc.dram_tensor("cc_in2", [SP, 128], F16, kind="Internal")
    u2full = nc.dram_tensor("u2full", [SP * NC_CORES, 128], F16,
                            kind="Internal", addr_space="Shared")

    nblk = [(int(sec_len[c]) + BLK - 1) // BLK for c in range(NCH)]
    nohg = [(int(sec_len[c]) + OHG - 1) // OHG for c in range(NCH)]

    with TileContext(nc) as tc:
        with (
            tc.tile_pool(name="const", bufs=1) as cpool,
            tc.tile_pool(name="stage", bufs=1) as spool,
            tc.tile_pool(name="msg", bufs=3) as mpool,
            tc.tile_pool(name="oh", bufs=3) as opool,
            tc.tile_pool(name="work", bufs=4) as wpool,
            tc.tile_pool(name="psA", bufs=3, space="PSUM") as psA,
            tc.tile_pool(name="psB", bufs=2, space="PSUM") as psB,
            tc.tile_pool(name="psC", bufs=2, space="PSUM") as psC,
        ):
            # ---- constants / streams ------------------------------------
            t_idx = cpool.tile([128, tot // 16], I16)
            nc.sync.dma_start(out=t_idx[:], in_=d_idx[:, :])
            t_rel = cpool.tile([128, tot // 128], F16)
            nc.sync.dma_start(out=t_rel[:], in_=d_rel[:, :])
            t_iota = cpool.tile([128, TS], F16)
            nc.sync.dma_start(out=t_iota[:], in_=d_iota[:, :])
            t_id32 = cpool.tile([128, 128], F32)
            nc.sync.dma_start(out=t_id32[:], in_=d_id32[:, :])
            t_id16 = cpool.tile([128, 128], F16)
            nc.sync.dma_start(out=t_id16[:], in_=d_id16[:, :])
            t_w1 = cpool.tile([FD, FD], F32)
            nc.sync.dma_start(out=t_w1[:], in_=d_w1[:, :])
            t_b1 = cpool.tile([128, FD], F32)
            nc.sync.dma_start(out=t_b1[:], in_=d_b1[:, :])
            t_w2 = cpool.tile([FD, OD], F32)
            nc.sync.dma_start(out=t_w2[:], in_=d_w2[:, :])
            t_b2 = cpool.tile([128, OD], F32)
            nc.sync.dma_start(out=t_b2[:], in_=d_b2[:, :])

            # ---- dinv ----------------------------------------------------
            t_deg = cpool.tile([128, TPC], F32)
            nc.sync.dma_start(out=t_deg[:], in_=d_deg[:, :])
            t_dinv = cpool.tile([128, TPC], F32)
            nc.vector.reciprocal(out=t_dinv[:], in_=t_deg[:])
            nc.scalar.activation(out=t_dinv[:], in_=t_dinv[:],
                                 func=mybir.ActivationFunctionType.Sqrt)

            # ---- u1 = dinv * x -> staging + allgather --------------------
            t_x = spool.tile([128, TPC, FD], F32)
            nc.sync.dma_start(out=t_x[:], in_=d_x[:, :, :])
            t_u1 = spool.tile([128, TPC, 128], F16)
            nc.vector.tensor_tensor(
                out=t_u1[:, :, 0:FD], in0=t_x[:],
                in1=t_dinv[:, :, None].to_broadcast([128, TPC, FD]),
                op=mybir.AluOpType.mult,
            )
            nc.sync.dma_start(
                out=cc_in1.rearrange("(t p) f -> p t f", p=128),
                in_=t_u1[:, :, :],
            )
            nc.gpsimd.collective_compute(
                "AllGather", mybir.AluOpType.bypass,
                ins=[cc_in1[:, :]], outs=[u1full[:, :]],
                replica_groups=[list(range(NC_CORES))],
            )

            # ---- one shared layer ---------------------------------------
            def layer(ufull, u_stage, w_tile, outd, epilogue):
                msg_tiles = {}
                oh_tiles = {}
                cursor_blk = [0] * NCH   # next block to emit per section
                cursor_ohg = [0] * NCH

                def ensure(c, upto_slot):
                    """Emit gather blocks / one-hot groups of section c
                    covering section-local slots < upto_slot."""
                    while cursor_blk[c] * BLK < upto_slot:
                        bi = cursor_blk[c]
                        ln = min(BLK, int(sec_len[c]) - bi * BLK)
                        blk = mpool.tile([128, BLK // 128, 128], F16,
                                         tag=f"msg{c}")
                        a = int(sec_base[c]) + bi * BLK
                        if _NO_GATHER:
                            nc.vector.memset(blk[:, 0:ln // 128, :], 0.0)
                        else:
                            nc.gpsimd.dma_gather(
                                blk[:, 0:ln // 128, :],
                                ufull[c * CH:(c + 1) * CH, :],
                                t_idx[:, a // 16:(a + ln) // 16],
                                ln, ln, 128,
                                single_packet=_SINGLE_PACKET,
                                queue_num=c,
                            )
                        msg_tiles[(c, bi)] = blk
                        cursor_blk[c] = bi + 1
                    while cursor_ohg[c] * OHG < upto_slot:
                        gi = cursor_ohg[c]
                        gl = min(OHG, int(sec_len[c]) - gi * OHG)
                        nb = gl // 128
                        ohp = opool.tile([128, OHG // 128, TS], F16,
                                         tag=f"oh{c}")
                        g0 = (int(sec_base[c]) + gi * OHG) // 128
                        nc.vector.tensor_tensor(
                            out=ohp[:, 0:nb, :],
                            in0=t_rel[:, g0:g0 + nb, None].to_broadcast(
                                [128, nb, TS]),
                            in1=t_iota[:, None, :].to_broadcast([128, nb, TS]),
                            op=mybir.AluOpType.is_equal,
                        )
                        oh_tiles[(c, gi)] = ohp
                        cursor_ohg[c] = gi + 1

                for t in range(TPC):
                    # emit data production for this tile's batches
                    for c in range(NCH):
                        rs = int(run_start[c, t] - sec_base[c])
                        rl = int(runs_padded[c, t])
                        if rl:
                            ensure(c, rs + rl)
                    # matmul group
                    ps = psA.tile([128, FD], F32, tag="agg")
                    nc.tensor.matmul(out=ps[:], lhsT=t_id16[:],
                                     rhs=u_stage[:, t, 0:FD],
                                     start=True, stop=False)
                    mms = []
                    for c in range(NCH):
                        rs = int(run_start[c, t] - sec_base[c])
                        rl = int(runs_padded[c, t])
                        for j in range(rl // 128):
                            g = rs + j * 128
                            mms.append((c, g))
                    for i, (c, g) in enumerate(mms):
                        oh = oh_tiles[(c, g // OHG)]
                        mg = msg_tiles[(c, g // BLK)]
                        nc.tensor.matmul(
                            out=ps[:],
                            lhsT=oh[:, (g % OHG) // 128, :],
                            rhs=mg[:, (g % BLK) // 128, 0:FD],
                            start=False, stop=(i == len(mms) - 1),
                        )
                    if not mms:
                        # close the accumulation group
                        nc.tensor.matmul(out=ps[:], lhsT=t_id16[:],
                                         rhs=u_stage[:, t, 0:FD],
                                         start=False, stop=True)
                        # note: this double-adds u_own; handled by epilogue?
                        # cannot happen: every tile has >= 1 batch in practice
                        raise AssertionError("tile with zero batches")
                    # epilogue: pre = dinv * agg; preT; out = preT.T @ W + b
                    pre = wpool.tile([128, FD], F32, tag="pre")
                    nc.vector.tensor_scalar(
                        out=pre[:], in0=ps[:], scalar1=t_dinv[:, t:t + 1],
                        scalar2=None, op0=mybir.AluOpType.mult,
                    )
                    pst = psB.tile([FD, 128], F32, tag="tr")
                    nc.tensor.transpose(out=pst[:], in_=pre[:],
                                        identity=t_id32[:])
                    preT = wpool.tile([FD, 128], F32, tag="preT")
                    nc.scalar.copy(out=preT[:], in_=pst[:])
                    po = psC.tile([128, outd], F32, tag="mm2")
                    nc.tensor.matmul(out=po[:], lhsT=preT[:], rhs=w_tile[:],
                                     start=True, stop=True)
                    epilogue(t, po)

            # ---- layer 1 -------------------------------------------------
            t_u2 = spool.tile([128, TPC, 128], F16)

            def epi1(t, po):
                xb = wpool.tile([128, FD], F32, tag="epi")
                nc.vector.tensor_tensor(out=xb[:], in0=po[:], in1=t_b1[:],
                                        op=mybir.AluOpType.add)
                nc.vector.tensor_scalar(
                    out=t_u2[:, t, 0:FD], in0=xb[:],
                    scalar1=0.0, scalar2=t_dinv[:, t:t + 1],
                    op0=mybir.AluOpType.max, op1=mybir.AluOpType.mult,
                )

            layer(u1full, t_u1, t_w1, FD, epi1)

            # ---- allgather u2 -------------------------------------------
            nc.sync.dma_start(
                out=cc_in2.rearrange("(t p) f -> p t f", p=128),
                in_=t_u2[:, :, :],
            )
            nc.gpsimd.collective_compute(
                "AllGather", mybir.AluOpType.bypass,
                ins=[cc_in2[:, :]], outs=[u2full[:, :]],
                replica_groups=[list(range(NC_CORES))],
            )

            # ---- layer 2 -------------------------------------------------
            def epi2(t, po):
                ob = wpool.tile([128, OD], F32, tag="epi")
                nc.vector.tensor_tensor(out=ob[:], in0=po[:], in1=t_b2[:],
                                        op=mybir.AluOpType.add)
                nc.sync.dma_start(out=d_out[:, t, :], in_=ob[:])

            layer(u2full, t_u2, t_w2, OD, epi2)

    nc.compile()
    return nc


# ---------------------------------------------------------------- entry point
def kernel(x, W1, b1, W2, b2, edge_index):
    x = np.asarray(x, dtype=np.float32)
    W1 = np.asarray(W1, dtype=np.float32)
    b1 = np.asarray(b1, dtype=np.float32)
    W2 = np.asarray(W2, dtype=np.float32)
    b2 = np.asarray(b2, dtype=np.float32)
    edge_index = np.asarray(edge_index)

    ekey = hash(edge_index.tobytes())
    if ekey in _compiled_cache:
        nc, meta = _compiled_cache[ekey]
    else:
        meta = _prep_edges(edge_index)
        runs_padded, run_start, sec_base, sec_len, tot, idx_s, rel_s = meta
        nc = _build(runs_padded, run_start, sec_base, sec_len, tot)
        _compiled_cache[ekey] = (nc, meta)
    runs_padded, run_start, sec_base, sec_len, tot, idx_s, rel_s = meta

    dst = np.asarray(edge_index[1], dtype=np.int64)
    deg_full = np.bincount(dst, minlength=N_NODES).astype(np.float32) + 1.0

    iota_np = np.tile(np.arange(TS, dtype=np.float16)[None, :], (128, 1))
    id32_np = np.eye(128, dtype=np.float32)
    id16_np = np.eye(128, dtype=np.float16)
    b1rep = np.tile(b1[None, :], (128, 1)).astype(np.float32)
    b2rep = np.tile(b2[None, :], (128, 1)).astype(np.float32)

    in_maps = []
    for k in range(NC_CORES):
        xs = np.zeros((SP, FD), dtype=np.float32)
        xs[:S] = x[k * S:(k + 1) * S]
        degs = np.ones((SP,), dtype=np.float32)
        degs[:S] = deg_full[k * S:(k + 1) * S]
        in_maps.append({
            "x_shard": xs.reshape(TPC, 128, FD).transpose(1, 0, 2).copy(),
            "deg_shard": degs.reshape(TPC, 128).T.copy(),
            "idx_stream": _wrap_idx(idx_s[k]),
            "rel_stream": _wrap_rel(rel_s[k]),
            "iota16": iota_np, "ident32": id32_np, "ident16": id16_np,
            "W1": W1, "b1rep": b1rep, "W2": W2, "b2rep": b2rep,
        })

    trace = bool(os.environ.get("BASS_TRACE"))
    res = run_bass_kernel_spmd(
        nc, in_maps, core_ids=list(range(NC_CORES)), trace=trace,
    )
    if trace and res.exec_time_ns is not None:
        print(f"HW exec time: {res.exec_time_ns} ns")
        kernel.last_exec_time_ns = res.exec_time_ns

    outs = []
    for k in range(NC_CORES):
        o = res.results[k]["out_shard"]          # [128, TPC, OD]
        outs.append(o.transpose(1, 0, 2).reshape(SP, OD)[:S])
    return np.concatenate(outs, axis=0)



# revision 2
# speedup vs baseline: 1.2417x; 1.2417x over previous
"""Bass/Trainium2 kernel for 2-layer GCN (nn_MeshGNN), 8 NeuronCores. v2.

Math (per layer, commuted form):
    A_hat = D^-1/2 (A+I) D^-1/2 ;  gcn(x) = A_hat x W + b
    u = dinv * x ;  agg[d] = sum_{e: dst=d} u[src[e]] + u[d]
    out = (dinv * agg) @ W + b           (layer 1 adds relu)

Distribution: nodes sharded by range across 8 cores (12500/core, padded to
12544 = 98 tiles of 128). The gather table u (fp16, 256B rows) is exchanged
in 4 row-BANDS via per-band AllGathers: band b holds tiles band_t0[b]:band_t1[b]
of EVERY core's shard, so chunk-c gathers unblock as soon as band c's
AllGather lands (layer 2's first AG fires mid-layer-1). The layer sweep is
chunk-major, accumulating per-tile partials in an SBUF f32 accumulator.
Edge streams (gather indices + relative-dst) are identical for both layers.
"""
import os
import numpy as np

import concourse.bacc as bacc
import concourse.mybir as mybir
from concourse.tile import TileContext
from concourse.bass_utils import run_bass_kernel_spmd

# ---------------------------------------------------------------- constants
N_NODES = 100000
NC_CORES = 8
S = 12500                 # nodes per core
TS = 128                  # dst-tile size
TPC = 98                  # dst tiles per core (98*128 = 12544)
SP = TPC * TS             # padded nodes per core
NCH = 4                   # src bands
BAND_TILES = [25, 25, 24, 24]
BAND_T0 = [sum(BAND_TILES[:i]) for i in range(NCH)]
BAND_T1 = [sum(BAND_TILES[:i + 1]) for i in range(NCH)]
CHR = [8 * 128 * bt for bt in BAND_TILES]   # table rows per band chunk
CHB = [sum(CHR[:i]) for i in range(NCH)]    # band chunk base row in big table
FD = 64                   # in/hidden feature dim
OD = 32                   # output dim
BLK = 1024                # gather block (slots per dma_gather, single-packet max)
OHG = 1024                # one-hot group (slots per DVE op)
F32 = mybir.dt.float32
F16 = mybir.dt.float16
I16 = mybir.dt.int16

_compiled_cache = {}


# ---------------------------------------------------------------- tile patch
def _install_tile_patch():
    """walrus here rejects >1 sync-wait on an InstDrain; split the Tile tail
    drain's waits across sequential drains (same engine => same semantics)."""
    from bass_rust import ScopedClock

    def _patched(self, tick_clock, wait_clock):
        drain_inst = self.nc.sync.drain()
        wait_clock.add_sem_waits(
            drain_inst.ins, ScopedClock({None: tick_clock.global_clock})
        )
        si = drain_inst.ins.sync_info
        waits = list(si.on_wait) if si and si.on_wait else []
        if len(waits) > 1:
            si.on_wait = waits[:1]
            for w in waits[1:]:
                extra = self.nc.sync.drain()
                extra.ins.sync_info = mybir.SyncInfo(on_wait=[w], on_update=[])
        self.nc.all_engine_barrier()
        assert self.sems is not None
        popped = self.nc._tile_sem_poison_stack.pop()
        assert popped is self._sem_poison
        self.nc.clear_and_free_semaphores(list(self.sems.allocated().values()))
        self.nc.all_engine_barrier()

    TileContext._drain_and_barrier = _patched


_install_tile_patch()


# ---------------------------------------------------------------- host prep
def _prep_edges(edge_index):
    """Shared-run-structure edge streams, band chunks. Returns (runs_padded
    [NCH,TPC], run_start [NCH,TPC], sec_base [NCH], sec_len [NCH], tot,
    idx_streams [NC][tot], rel_streams [NC][tot])."""
    src = np.asarray(edge_index[0], dtype=np.int64)
    dst = np.asarray(edge_index[1], dtype=np.int64)

    # source table row within its band chunk
    k = src // S
    pos = src % S
    stile = pos // TS
    band_of_tile = np.zeros(TPC, dtype=np.int64)
    for b in range(NCH):
        band_of_tile[BAND_T0[b]:BAND_T1[b]] = b
    band = band_of_tile[stile]
    t0 = np.asarray(BAND_T0, dtype=np.int64)
    trow = k * (np.asarray(BAND_TILES)[band] * TS) \
        + (stile - t0[band]) * TS + pos % TS

    core = dst // S
    dstloc = dst - core * S
    tile = dstloc // TS

    key = (core * NCH + band) * TPC + tile
    counts = np.bincount(key, minlength=NC_CORES * NCH * TPC).reshape(
        NC_CORES, NCH, TPC
    )
    runs = counts.max(axis=0)                       # [NCH, TPC]
    runs_padded = ((runs + TS - 1) // TS) * TS      # mult of 128
    sec_len = runs_padded.sum(axis=1)               # [NCH]
    # pad each section to BLK multiple so gather blocks stay in-section
    sec_len = ((sec_len + BLK - 1) // BLK) * BLK
    sec_base = np.concatenate([[0], np.cumsum(sec_len)[:-1]])
    run_start = sec_base[:, None] + (
        np.cumsum(runs_padded, axis=1) - runs_padded
    )                                               # [NCH, TPC] global start
    tot = int(sec_len.sum())

    idx_streams, rel_streams = [], []
    for kk in range(NC_CORES):
        sel = core == kk
        c_k, t_k = band[sel], tile[sel]
        row_k = trow[sel]
        rel_k = (dstloc[sel] - t_k * TS).astype(np.float16)
        # within each (band, tile) run, order slots by ascending table row:
        # the gather's random reads become monotonic (DRAM-friendlier)
        order = np.lexsort((row_k, t_k, c_k))
        c_k, t_k, row_k, rel_k = c_k[order], t_k[order], row_k[order], rel_k[order]
        key_k = c_k * TPC + t_k
        cnt_k = np.bincount(key_k, minlength=NCH * TPC)
        grp_start = np.cumsum(cnt_k) - cnt_k
        within = np.arange(len(key_k)) - grp_start[key_k]
        slot = run_start.reshape(-1)[key_k] + within
        idx_s = np.zeros(tot, dtype=np.int16)
        rel_s = np.full(tot, -1.0, dtype=np.float16)
        idx_s[slot] = row_k.astype(np.int16)
        rel_s[slot] = rel_k
        idx_streams.append(idx_s)
        rel_streams.append(rel_s)
    return runs_padded, run_start, sec_base, sec_len, tot, idx_streams, rel_streams


def _wrap_idx(idx_s):
    """[tot] int16 -> [128, tot/16] wrapped + replicated across 8 groups."""
    tot = idx_s.shape[0]
    w = idx_s.reshape(tot // 16, 16).T              # [16, tot/16]
    return np.tile(w, (8, 1)).copy()                # [128, tot/16]


def _wrap_rel(rel_s):
    tot = rel_s.shape[0]
    return rel_s.reshape(tot // 128, 128).T.copy()  # [128, tot/128]


# ---------------------------------------------------------------- kernel build
def _build(runs_padded, run_start, sec_base, sec_len, tot):
    nc = bacc.Bacc(None, target_bir_lowering=False, debug=False,
                   num_devices=NC_CORES, num_swdge_queues=4)

    # ---- I/O -------------------------------------------------------------
    d_x = nc.dram_tensor("x_shard", [128, TPC, FD], F32, kind="ExternalInput")
    d_deg = nc.dram_tensor("deg_shard", [128, TPC], F32, kind="ExternalInput")
    d_idx = nc.dram_tensor("idx_stream", [128, tot // 16], I16, kind="ExternalInput")
    d_rel = nc.dram_tensor("rel_stream", [128, tot // 128], F16, kind="ExternalInput")
    d_iota = nc.dram_tensor("iota16", [128, TS], F16, kind="ExternalInput")
    d_id32 = nc.dram_tensor("ident32", [128, 128], F32, kind="ExternalInput")
    d_id16 = nc.dram_tensor("ident16", [128, 128], F16, kind="ExternalInput")
    d_w1 = nc.dram_tensor("W1", [FD, FD], F32, kind="ExternalInput")
    d_b1 = nc.dram_tensor("b1rep", [128, FD], F32, kind="ExternalInput")
    d_w2 = nc.dram_tensor("W2", [FD, OD], F32, kind="ExternalInput")
    d_b2 = nc.dram_tensor("b2rep", [128, OD], F32, kind="ExternalInput")
    d_out = nc.dram_tensor("out_shard", [128, TPC, OD], F32, kind="ExternalOutput")

    cc_in1 = nc.dram_tensor("cc_in1", [SP, 128], F16, kind="Internal")
    cc_in2 = nc.dram_tensor("cc_in2", [SP, 128], F16, kind="Internal")
    u1big = nc.dram_tensor("u1full", [SP * NC_CORES, 128], F16,
                           kind="Internal", addr_space="Shared")
    u2big = nc.dram_tensor("u2full", [SP * NC_CORES, 128], F16,
                           kind="Internal", addr_space="Shared")
    u1full = [u1big[CHB[c]:CHB[c] + CHR[c], :] for c in range(NCH)]
    u2full = [u2big[CHB[c]:CHB[c] + CHR[c], :] for c in range(NCH)]

    with TileContext(nc) as tc:
        with (
            tc.tile_pool(name="const", bufs=1) as cpool,
            tc.tile_pool(name="stage", bufs=1) as spool,
            tc.tile_pool(name="msg", bufs=7) as mpool,
            tc.tile_pool(name="oh", bufs=3) as opool,
            tc.tile_pool(name="work", bufs=4) as wpool,
            tc.tile_pool(name="psA", bufs=4, space="PSUM") as psA,
            tc.tile_pool(name="psB", bufs=2, space="PSUM") as psB,
            tc.tile_pool(name="psC", bufs=2, space="PSUM") as psC,
        ):
            # ---- constants / streams ------------------------------------
            t_idx = cpool.tile([128, tot // 16], I16)
            nc.sync.dma_start(out=t_idx[:], in_=d_idx[:, :])
            t_rel = cpool.tile([128, tot // 128], F16)
            nc.sync.dma_start(out=t_rel[:], in_=d_rel[:, :])
            t_iota = cpool.tile([128, TS], F16)
            nc.sync.dma_start(out=t_iota[:], in_=d_iota[:, :])
            t_id32 = cpool.tile([128, 128], F32)
            nc.sync.dma_start(out=t_id32[:], in_=d_id32[:, :])
            t_id16 = cpool.tile([128, 128], F16)
            nc.sync.dma_start(out=t_id16[:], in_=d_id16[:, :])
            t_w1 = cpool.tile([FD, FD], F32)
            nc.sync.dma_start(out=t_w1[:], in_=d_w1[:, :])
            t_b1 = cpool.tile([128, FD], F32)
            nc.sync.dma_start(out=t_b1[:], in_=d_b1[:, :])
            t_w2 = cpool.tile([FD, OD], F32)
            nc.sync.dma_start(out=t_w2[:], in_=d_w2[:, :])
            t_b2 = cpool.tile([128, OD], F32)
            nc.sync.dma_start(out=t_b2[:], in_=d_b2[:, :])

            # ---- dinv ----------------------------------------------------
            t_deg = cpool.tile([128, TPC], F32)
            nc.sync.dma_start(out=t_deg[:], in_=d_deg[:, :])
            t_dinv = cpool.tile([128, TPC], F32)
            nc.vector.reciprocal(out=t_dinv[:], in_=t_deg[:])
            nc.scalar.activation(out=t_dinv[:], in_=t_dinv[:],
                                 func=mybir.ActivationFunctionType.Sqrt)

            # ---- u1 = dinv * x -> staging + band AllGathers --------------
            t_x = spool.tile([128, TPC, FD], F32)
            nc.sync.dma_start(out=t_x[:], in_=d_x[:, :, :])
            t_u1 = spool.tile([128, TPC, FD], F16)
            nc.vector.tensor_tensor(
                out=t_u1[:, :, :], in0=t_x[:],
                in1=t_dinv[:, :, None].to_broadcast([128, TPC, FD]),
                op=mybir.AluOpType.mult,
            )
            cc1_view = cc_in1.rearrange("(t p) f -> p t f", p=128)
            nc.sync.dma_start(out=cc1_view[:, :, 0:FD], in_=t_u1[:, :, :])
            # single AG into band-chunk layout: out rows grouped band-major
            # via a strided view [band pieces].
            for c in range(NCH):
                nc.gpsimd.collective_compute(
                    "AllGather", mybir.AluOpType.bypass,
                    ins=[cc_in1[BAND_T0[c] * TS:BAND_T1[c] * TS, :]],
                    outs=[u1full[c]],
                    replica_groups=[list(range(NC_CORES))],
                )

            # ---- one shared layer (tile-major sweep, band AG overlap) ----
            t_u2 = spool.tile([128, TPC, FD], F16)

            def layer(ufull, u_stage, w_tile, outd, epilogue, post_band,
                      prefix=0):
                msg_tiles = {}
                oh_tiles = {}
                cursor_blk = [0] * NCH
                cursor_ohg = [0] * NCH

                def ensure(c, upto_slot):
                    while cursor_blk[c] * BLK < upto_slot:
                        bi = cursor_blk[c]
                        ln = min(BLK, int(sec_len[c]) - bi * BLK)
                        blk = mpool.tile([128, BLK // 128, 128], F16,
                                         tag=f"msg{c}")
                        a = int(sec_base[c]) + bi * BLK
                        nc.gpsimd.dma_gather(
                            blk[:, 0:ln // 128, :],
                            ufull[c],
                            t_idx[:, a // 16:(a + ln) // 16],
                            ln, ln, 128,
                            single_packet=True,
                            queue_num=c % 4,
                        )
                        msg_tiles[(c, bi)] = blk
                        cursor_blk[c] = bi + 1
                    while cursor_ohg[c] * OHG < upto_slot:
                        gi = cursor_ohg[c]
                        gl = min(OHG, int(sec_len[c]) - gi * OHG)
                        nb = gl // 128
                        ohp = opool.tile([128, OHG // 128, TS], F16,
                                         tag=f"oh{c}")
                        g0 = (int(sec_base[c]) + gi * OHG) // 128
                        nc.vector.tensor_tensor(
                            out=ohp[:, 0:nb, :],
                            in0=t_rel[:, g0:g0 + nb, None].to_broadcast(
                                [128, nb, TS]),
                            in1=t_iota[:, None, :].to_broadcast([128, nb, TS]),
                            op=mybir.AluOpType.is_equal,
                        )
                        oh_tiles[(c, gi)] = ohp
                        cursor_ohg[c] = gi + 1

                if prefix:
                    for c in range(NCH - 1):
                        ensure(c, prefix * BLK)
                for t in range(TPC):
                    for c in range(NCH):
                        rs = int(run_start[c, t] - sec_base[c])
                        rl = int(runs_padded[c, t])
                        if rl:
                            ensure(c, rs + rl)
                    ps = psA.tile([128, FD], F32, tag="agg")
                    nc.tensor.matmul(out=ps[:], lhsT=t_id16[:],
                                     rhs=u_stage[:, t, :],
                                     start=True, stop=False)
                    mms = []
                    for c in range(NCH):
                        rs = int(run_start[c, t] - sec_base[c])
                        rl = int(runs_padded[c, t])
                        for j in range(rl // 128):
                            g = rs + j * 128
                            mms.append((c, g))
                    for i, (c, g) in enumerate(mms):
                        oh = oh_tiles[(c, g // OHG)]
                        mg = msg_tiles[(c, g // BLK)]
                        nc.tensor.matmul(
                            out=ps[:],
                            lhsT=oh[:, (g % OHG) // 128, :],
                            rhs=mg[:, (g % BLK) // 128, 0:FD],
                            start=False, stop=(i == len(mms) - 1),
                        )
                    assert mms
                    # epilogue: pre = dinv * agg; preT; po = preT.T @ W
                    pre = wpool.tile([128, FD], F32, tag="pre")
                    nc.vector.tensor_scalar(
                        out=pre[:], in0=ps[:], scalar1=t_dinv[:, t:t + 1],
                        scalar2=None, op0=mybir.AluOpType.mult,
                    )
                    pst = psB.tile([FD, 128], F32, tag="tr")
                    nc.tensor.transpose(out=pst[:], in_=pre[:],
                                        identity=t_id32[:])
                    preT = wpool.tile([FD, 128], F32, tag="preT")
                    nc.scalar.copy(out=preT[:], in_=pst[:])
                    po = psC.tile([128, outd], F32, tag="mm2")
                    nc.tensor.matmul(out=po[:], lhsT=preT[:], rhs=w_tile[:],
                                     start=True, stop=True)
                    epilogue(t, po)
                    for b in range(NCH):
                        if t == BAND_T1[b] - 1:
                            post_band(b)

            # ---- layer 1 -------------------------------------------------
            def epi1(t, po):
                xb = wpool.tile([128, FD], F32, tag="epi")
                nc.vector.tensor_tensor(out=xb[:], in0=po[:], in1=t_b1[:],
                                        op=mybir.AluOpType.add)
                nc.scalar.activation(
                    out=t_u2[:, t, :], in_=xb[:],
                    func=mybir.ActivationFunctionType.Relu,
                    scale=t_dinv[:, t:t + 1],
                )

            cc2_view = cc_in2.rearrange("(t p) f -> p t f", p=128)

            def post_band(b):
                nc.sync.dma_start(
                    out=cc2_view[:, BAND_T0[b]:BAND_T1[b], 0:FD],
                    in_=t_u2[:, BAND_T0[b]:BAND_T1[b], :],
                )
                nc.gpsimd.collective_compute(
                    "AllGather", mybir.AluOpType.bypass,
                    ins=[cc_in2[BAND_T0[b] * TS:BAND_T1[b] * TS, :]],
                    outs=[u2full[b]],
                    replica_groups=[list(range(NC_CORES))],
                )

            layer(u1full, t_u1, t_w1, FD, epi1, post_band, prefix=5)

            # ---- layer 2 -------------------------------------------------
            def epi2(t, po):
                ob = wpool.tile([128, OD], F32, tag="epi")
                nc.vector.tensor_tensor(out=ob[:], in0=po[:], in1=t_b2[:],
                                        op=mybir.AluOpType.add)
                nc.sync.dma_start(out=d_out[:, t, :], in_=ob[:])

            layer(u2full, t_u2, t_w2, OD, epi2, lambda b: None,
                  prefix=5)

    nc.compile()
    return nc


# ---------------------------------------------------------------- entry point
def kernel(x, W1, b1, W2, b2, edge_index):
    x = np.asarray(x, dtype=np.float32)
    W1 = np.asarray(W1, dtype=np.float32)
    b1 = np.asarray(b1, dtype=np.float32)
    W2 = np.asarray(W2, dtype=np.float32)
    b2 = np.asarray(b2, dtype=np.float32)
    edge_index = np.asarray(edge_index)

    ekey = hash(edge_index.tobytes())
    if ekey in _compiled_cache:
        nc, meta = _compiled_cache[ekey]
    else:
        meta = _prep_edges(edge_index)
        runs_padded, run_start, sec_base, sec_len, tot, idx_s, rel_s = meta
        nc = _build(runs_padded, run_start, sec_base, sec_len, tot)
        _compiled_cache[ekey] = (nc, meta)
    runs_padded, run_start, sec_base, sec_len, tot, idx_s, rel_s = meta

    dst = np.asarray(edge_index[1], dtype=np.int64)
    deg_full = np.bincount(dst, minlength=N_NODES).astype(np.float32) + 1.0

    iota_np = np.tile(np.arange(TS, dtype=np.float16)[None, :], (128, 1))
    id32_np = np.eye(128, dtype=np.float32)
    id16_np = np.eye(128, dtype=np.float16)
    b1rep = np.tile(b1[None, :], (128, 1)).astype(np.float32)
    b2rep = np.tile(b2[None, :], (128, 1)).astype(np.float32)

    in_maps = []
    for k in range(NC_CORES):
        xs = np.zeros((SP, FD), dtype=np.float32)
        xs[:S] = x[k * S:(k + 1) * S]
        degs = np.ones((SP,), dtype=np.float32)
        degs[:S] = deg_full[k * S:(k + 1) * S]
        in_maps.append({
            "x_shard": xs.reshape(TPC, 128, FD).transpose(1, 0, 2).copy(),
            "deg_shard": degs.reshape(TPC, 128).T.copy(),
            "idx_stream": _wrap_idx(idx_s[k]),
            "rel_stream": _wrap_rel(rel_s[k]),
            "iota16": iota_np, "ident32": id32_np, "ident16": id16_np,
            "W1": W1, "b1rep": b1rep, "W2": W2, "b2rep": b2rep,
        })

    trace = bool(os.environ.get("BASS_TRACE"))
    res = run_bass_kernel_spmd(
        nc, in_maps, core_ids=list(range(NC_CORES)), trace=trace,
    )
    if trace and res.exec_time_ns is not None:
        print(f"HW exec time: {res.exec_time_ns} ns")
        kernel.last_exec_time_ns = res.exec_time_ns

    outs = []
    for k in range(NC_CORES):
        o = res.results[k]["out_shard"]          # [128, TPC, OD]
        outs.append(o.transpose(1, 0, 2).reshape(SP, OD)[:S])
    return np.concatenate(outs, axis=0)


# revision 3
# speedup vs baseline: 1.4606x; 1.1763x over previous
"""Bass/Trainium2 kernel for 2-layer GCN (nn_MeshGNN), 8 NeuronCores. v2.

Math (per layer, commuted form):
    A_hat = D^-1/2 (A+I) D^-1/2 ;  gcn(x) = A_hat x W + b
    u = dinv * x ;  agg[d] = sum_{e: dst=d} u[src[e]] + u[d]
    out = (dinv * agg) @ W + b           (layer 1 adds relu)

Distribution: nodes sharded by range across 8 cores (12500/core, padded to
12544 = 98 tiles of 128). The gather table u (fp16, 256B rows) is exchanged
in 4 row-BANDS via per-band AllGathers: band b holds tiles band_t0[b]:band_t1[b]
of EVERY core's shard, so chunk-c gathers unblock as soon as band c's
AllGather lands (layer 2's first AG fires mid-layer-1). The layer sweep is
chunk-major, accumulating per-tile partials in an SBUF f32 accumulator.
Edge streams (gather indices + relative-dst) are identical for both layers.
"""
import os
import numpy as np

import concourse.bacc as bacc
import concourse.mybir as mybir
from concourse.tile import TileContext
from concourse.bass_utils import run_bass_kernel_spmd

# ---------------------------------------------------------------- constants
N_NODES = 100000
NC_CORES = 8
S = 12500                 # nodes per core
TS = 128                  # dst-tile size
TPC = 98                  # dst tiles per core (98*128 = 12544)
SP = TPC * TS             # padded nodes per core
NCH = 4                   # src bands
BAND_TILES = [25, 25, 24, 24]
BAND_T0 = [sum(BAND_TILES[:i]) for i in range(NCH)]
BAND_T1 = [sum(BAND_TILES[:i + 1]) for i in range(NCH)]
CHR = [8 * 128 * bt for bt in BAND_TILES]   # table rows per band chunk
CHB = [sum(CHR[:i]) for i in range(NCH)]    # band chunk base row in big table
FD = 64                   # in/hidden feature dim
OD = 32                   # output dim
BLK = 1024                # gather block (slots per dma_gather, single-packet max)
OHG = 1024                # one-hot group (slots per DVE op)
F32 = mybir.dt.float32
F16 = mybir.dt.float16
I16 = mybir.dt.int16

_compiled_cache = {}


# ---------------------------------------------------------------- tile patch
def _install_tile_patch():
    """walrus here rejects >1 sync-wait on an InstDrain; split the Tile tail
    drain's waits across sequential drains (same engine => same semantics)."""
    from bass_rust import ScopedClock

    def _patched(self, tick_clock, wait_clock):
        drain_inst = self.nc.sync.drain()
        wait_clock.add_sem_waits(
            drain_inst.ins, ScopedClock({None: tick_clock.global_clock})
        )
        si = drain_inst.ins.sync_info
        waits = list(si.on_wait) if si and si.on_wait else []
        if len(waits) > 1:
            si.on_wait = waits[:1]
            for w in waits[1:]:
                extra = self.nc.sync.drain()
                extra.ins.sync_info = mybir.SyncInfo(on_wait=[w], on_update=[])
        self.nc.all_engine_barrier()
        assert self.sems is not None
        popped = self.nc._tile_sem_poison_stack.pop()
        assert popped is self._sem_poison
        self.nc.clear_and_free_semaphores(list(self.sems.allocated().values()))
        self.nc.all_engine_barrier()

    TileContext._drain_and_barrier = _patched


_install_tile_patch()


# ---------------------------------------------------------------- host prep
def _prep_edges(edge_index):
    """Shared-run-structure edge streams, band chunks. Returns (runs_padded
    [NCH,TPC], run_start [NCH,TPC], sec_base [NCH], sec_len [NCH], tot,
    idx_streams [NC][tot], rel_streams [NC][tot])."""
    src = np.asarray(edge_index[0], dtype=np.int64)
    dst = np.asarray(edge_index[1], dtype=np.int64)

    # source table row within its band chunk
    k = src // S
    pos = src % S
    stile = pos // TS
    band_of_tile = np.zeros(TPC, dtype=np.int64)
    for b in range(NCH):
        band_of_tile[BAND_T0[b]:BAND_T1[b]] = b
    band = band_of_tile[stile]
    t0 = np.asarray(BAND_T0, dtype=np.int64)
    trow = k * (np.asarray(BAND_TILES)[band] * TS) \
        + (stile - t0[band]) * TS + pos % TS

    core = dst // S
    dstloc = dst - core * S
    tile = dstloc // TS

    key = (core * NCH + band) * TPC + tile
    counts = np.bincount(key, minlength=NC_CORES * NCH * TPC).reshape(
        NC_CORES, NCH, TPC
    )
    runs = counts.max(axis=0)                       # [NCH, TPC] exact max
    sec_len = runs.sum(axis=1)
    # pad each section to BLK multiple so gather blocks stay in-section
    sec_len = ((sec_len + BLK - 1) // BLK) * BLK
    sec_base = np.concatenate([[0], np.cumsum(sec_len)[:-1]])
    run_start = sec_base[:, None] + (np.cumsum(runs, axis=1) - runs)
    tot = int(sec_len.sum())

    # per-(c,t) matmul list: columns of 128 slots overlapping the run;
    # per-mm one-hot source = rel values masked to the run's slot range.
    tile_mms = [[None] * TPC for _ in range(NCH)]
    sec_mm_len = np.zeros(NCH, dtype=np.int64)
    mm_slot = []       # global mm -> (slot_lo, slot_hi, col_base)
    for c in range(NCH):
        m0 = 0
        for t in range(TPC):
            a = int(run_start[c, t]) - int(sec_base[c])
            b = a + int(runs[c, t])
            cols = range(a // TS, (b - 1) // TS + 1) if b > a else []
            tile_mms[c][t] = [(m0 + i, j) for i, j in enumerate(cols)]
            for j in cols:
                mm_slot.append((c, a, b, j * TS))
            m0 += len(tile_mms[c][t])
        sec_mm_len[c] = m0
    mm_tot = int(sec_mm_len.sum())
    mm_tot_pad = ((mm_tot + 7) // 8) * 8

    idx_streams, rel_streams = [], []
    for kk in range(NC_CORES):
        sel = core == kk
        c_k, t_k = band[sel], tile[sel]
        row_k = trow[sel]
        rel_k = (dstloc[sel] - t_k * TS).astype(np.float16)
        # within each (band, tile) run, order slots by ascending table row:
        # the gather's random reads become monotonic (DRAM-friendlier)
        order = np.lexsort((row_k, t_k, c_k))
        c_k, t_k, row_k, rel_k = c_k[order], t_k[order], row_k[order], rel_k[order]
        key_k = c_k * TPC + t_k
        cnt_k = np.bincount(key_k, minlength=NCH * TPC)
        grp_start = np.cumsum(cnt_k) - cnt_k
        within = np.arange(len(key_k)) - grp_start[key_k]
        slot = run_start.reshape(-1)[key_k] + within
        idx_s = np.zeros(tot, dtype=np.int16)
        rel_s = np.full(tot, -1.0, dtype=np.float16)
        idx_s[slot] = row_k.astype(np.int16)
        rel_s[slot] = rel_k
        # per-mm masked rel stream [mm_tot_pad * 128]
        relmm = np.full(mm_tot_pad * TS, -1.0, dtype=np.float16)
        for m, (c, a, b, col0) in enumerate(mm_slot):
            lo = max(a, col0)
            hi = min(b, col0 + TS)
            src = rel_s[int(sec_base[c]) + lo:int(sec_base[c]) + hi]
            relmm[m * TS + (lo - col0):m * TS + (hi - col0)] = src
        idx_streams.append(idx_s)
        rel_streams.append(relmm)
    return (runs, run_start, sec_base, sec_len, tot, idx_streams, rel_streams,
            tile_mms, sec_mm_len, mm_tot_pad)


def _wrap_idx(idx_s):
    """[tot] int16 -> [128, tot/16] wrapped + replicated across 8 groups."""
    tot = idx_s.shape[0]
    w = idx_s.reshape(tot // 16, 16).T              # [16, tot/16]
    return np.tile(w, (8, 1)).copy()                # [128, tot/16]


def _wrap_rel(rel_s):
    tot = rel_s.shape[0]
    return rel_s.reshape(tot // 128, 128).T.copy()  # [128, tot/128]


# ---------------------------------------------------------------- kernel build
def _build(runs, run_start, sec_base, sec_len, tot, tile_mms, sec_mm_len,
           mm_tot_pad):
    sec_mm_base = np.concatenate([[0], np.cumsum(sec_mm_len)[:-1]])
    nc = bacc.Bacc(None, target_bir_lowering=False, debug=False,
                   num_devices=NC_CORES, num_swdge_queues=4)

    # ---- I/O -------------------------------------------------------------
    d_x = nc.dram_tensor("x_shard", [128, TPC, FD], F32, kind="ExternalInput")
    d_deg = nc.dram_tensor("deg_shard", [128, TPC], F32, kind="ExternalInput")
    d_idx = nc.dram_tensor("idx_stream", [128, tot // 16], I16, kind="ExternalInput")
    d_rel = nc.dram_tensor("rel_stream", [128, mm_tot_pad], F16, kind="ExternalInput")
    d_iota = nc.dram_tensor("iota16", [128, TS], F16, kind="ExternalInput")
    d_id32 = nc.dram_tensor("ident32", [128, 128], F32, kind="ExternalInput")
    d_id16 = nc.dram_tensor("ident16", [128, 128], F16, kind="ExternalInput")
    d_w1 = nc.dram_tensor("W1", [FD, FD], F32, kind="ExternalInput")
    d_b1 = nc.dram_tensor("b1rep", [128, FD], F32, kind="ExternalInput")
    d_w2 = nc.dram_tensor("W2", [FD, OD], F32, kind="ExternalInput")
    d_b2 = nc.dram_tensor("b2rep", [128, OD], F32, kind="ExternalInput")
    d_out = nc.dram_tensor("out_shard", [128, TPC, OD], F32, kind="ExternalOutput")

    cc_in1 = nc.dram_tensor("cc_in1", [SP, 128], F16, kind="Internal")
    cc_in2 = nc.dram_tensor("cc_in2", [SP, 128], F16, kind="Internal")
    u1big = nc.dram_tensor("u1full", [SP * NC_CORES, 128], F16,
                           kind="Internal", addr_space="Shared")
    u2big = nc.dram_tensor("u2full", [SP * NC_CORES, 128], F16,
                           kind="Internal", addr_space="Shared")
    u1full = [u1big[CHB[c]:CHB[c] + CHR[c], :] for c in range(NCH)]
    u2full = [u2big[CHB[c]:CHB[c] + CHR[c], :] for c in range(NCH)]

    with TileContext(nc) as tc:
        with (
            tc.tile_pool(name="const", bufs=1) as cpool,
            tc.tile_pool(name="stage", bufs=1) as spool,
            tc.tile_pool(name="msg", bufs=7) as mpool,
            tc.tile_pool(name="oh", bufs=3) as opool,
            tc.tile_pool(name="work", bufs=4) as wpool,
            tc.tile_pool(name="psA", bufs=4, space="PSUM") as psA,
            tc.tile_pool(name="psB", bufs=2, space="PSUM") as psB,
            tc.tile_pool(name="psC", bufs=2, space="PSUM") as psC,
        ):
            # ---- constants / streams ------------------------------------
            t_idx = cpool.tile([128, tot // 16], I16)
            nc.sync.dma_start(out=t_idx[:], in_=d_idx[:, :])
            t_rel = cpool.tile([128, mm_tot_pad], F16)
            nc.sync.dma_start(out=t_rel[:], in_=d_rel[:, :])
            t_iota = cpool.tile([128, TS], F16)
            nc.sync.dma_start(out=t_iota[:], in_=d_iota[:, :])
            t_id32 = cpool.tile([128, 128], F32)
            nc.sync.dma_start(out=t_id32[:], in_=d_id32[:, :])
            t_id16 = cpool.tile([128, 128], F16)
            nc.sync.dma_start(out=t_id16[:], in_=d_id16[:, :])
            t_w1 = cpool.tile([FD, FD], F32)
            nc.sync.dma_start(out=t_w1[:], in_=d_w1[:, :])
            t_b1 = cpool.tile([128, FD], F32)
            nc.sync.dma_start(out=t_b1[:], in_=d_b1[:, :])
            t_w2 = cpool.tile([FD, OD], F32)
            nc.sync.dma_start(out=t_w2[:], in_=d_w2[:, :])
            t_b2 = cpool.tile([128, OD], F32)
            nc.sync.dma_start(out=t_b2[:], in_=d_b2[:, :])

            # ---- dinv ----------------------------------------------------
            t_deg = cpool.tile([128, TPC], F32)
            nc.sync.dma_start(out=t_deg[:], in_=d_deg[:, :])
            t_dinv = cpool.tile([128, TPC], F32)
            nc.vector.reciprocal(out=t_dinv[:], in_=t_deg[:])
            nc.scalar.activation(out=t_dinv[:], in_=t_dinv[:],
                                 func=mybir.ActivationFunctionType.Sqrt)

            # ---- u1 = dinv * x -> staging + band AllGathers --------------
            t_x = spool.tile([128, TPC, FD], F32)
            nc.sync.dma_start(out=t_x[:], in_=d_x[:, :, :])
            t_u1 = spool.tile([128, TPC, FD], F16)
            nc.vector.tensor_tensor(
                out=t_u1[:, :, :], in0=t_x[:],
                in1=t_dinv[:, :, None].to_broadcast([128, TPC, FD]),
                op=mybir.AluOpType.mult,
            )
            cc1_view = cc_in1.rearrange("(t p) f -> p t f", p=128)
            nc.sync.dma_start(out=cc1_view[:, :, 0:FD], in_=t_u1[:, :, :])
            # single AG into band-chunk layout: out rows grouped band-major
            # via a strided view [band pieces].
            for c in range(NCH):
                nc.gpsimd.collective_compute(
                    "AllGather", mybir.AluOpType.bypass,
                    ins=[cc_in1[BAND_T0[c] * TS:BAND_T1[c] * TS, :]],
                    outs=[u1full[c]],
                    replica_groups=[list(range(NC_CORES))],
                )

            # ---- one shared layer (tile-major sweep, band AG overlap) ----
            t_u2 = spool.tile([128, TPC, FD], F16)

            def layer(ufull, u_stage, w_tile, outd, epilogue, post_band,
                      prefix=0):
                msg_tiles = {}
                oh_tiles = {}
                cursor_blk = [0] * NCH
                cursor_ohg = [0] * NCH

                def ensure(c, upto_slot, upto_mm):
                    while cursor_blk[c] * BLK < upto_slot:
                        bi = cursor_blk[c]
                        ln = min(BLK, int(sec_len[c]) - bi * BLK)
                        blk = mpool.tile([128, BLK // 128, 128], F16,
                                         tag=f"msg{c}")
                        a = int(sec_base[c]) + bi * BLK
                        nc.gpsimd.dma_gather(
                            blk[:, 0:ln // 128, :],
                            ufull[c],
                            t_idx[:, a // 16:(a + ln) // 16],
                            ln, ln, 128,
                            single_packet=True,
                            queue_num=c % 4,
                        )
                        msg_tiles[(c, bi)] = blk
                        cursor_blk[c] = bi + 1
                    while cursor_ohg[c] * 8 < upto_mm:
                        gi = cursor_ohg[c]
                        nb = min(8, int(sec_mm_len[c]) - gi * 8)
                        ohp = opool.tile([128, 8, TS], F16, tag=f"oh{c}")
                        g0 = int(sec_mm_base[c]) + gi * 8
                        nc.vector.tensor_tensor(
                            out=ohp[:, 0:nb, :],
                            in0=t_rel[:, g0:g0 + nb, None].to_broadcast(
                                [128, nb, TS]),
                            in1=t_iota[:, None, :].to_broadcast([128, nb, TS]),
                            op=mybir.AluOpType.is_equal,
                        )
                        oh_tiles[(c, gi)] = ohp
                        cursor_ohg[c] = gi + 1

                if prefix:
                    for c in range(NCH - 1):
                        ensure(c, prefix * BLK, prefix * 8)
                for t in range(TPC):
                    for c in range(NCH):
                        rs = int(run_start[c, t] - sec_base[c])
                        rl = int(runs[c, t])
                        if tile_mms[c][t]:
                            ensure(c, rs + rl, tile_mms[c][t][-1][0] + 1)
                    ps = psA.tile([128, FD], F32, tag="agg")
                    nc.tensor.matmul(out=ps[:], lhsT=t_id16[:],
                                     rhs=u_stage[:, t, :],
                                     start=True, stop=False)
                    mms = []
                    for c in range(NCH):
                        for m, j in tile_mms[c][t]:
                            mms.append((c, m, j))
                    for i, (c, m, j) in enumerate(mms):
                        oh = oh_tiles[(c, m // 8)]
                        mg = msg_tiles[(c, j * TS // BLK)]
                        nc.tensor.matmul(
                            out=ps[:],
                            lhsT=oh[:, m % 8, :],
                            rhs=mg[:, (j * TS % BLK) // 128, 0:FD],
                            start=False, stop=(i == len(mms) - 1),
                        )
                    assert mms
                    # epilogue: pre = dinv * agg; preT; po = preT.T @ W
                    pre = wpool.tile([128, FD], F32, tag="pre")
                    nc.vector.tensor_scalar(
                        out=pre[:], in0=ps[:], scalar1=t_dinv[:, t:t + 1],
                        scalar2=None, op0=mybir.AluOpType.mult,
                    )
                    pst = psB.tile([FD, 128], F32, tag="tr")
                    nc.tensor.transpose(out=pst[:], in_=pre[:],
                                        identity=t_id32[:])
                    preT = wpool.tile([FD, 128], F32, tag="preT")
                    nc.scalar.copy(out=preT[:], in_=pst[:])
                    po = psC.tile([128, outd], F32, tag="mm2")
                    nc.tensor.matmul(out=po[:], lhsT=preT[:], rhs=w_tile[:],
                                     start=True, stop=True)
                    epilogue(t, po)
                    for b in range(NCH):
                        if t == BAND_T1[b] - 1:
                            post_band(b)

            # ---- layer 1 -------------------------------------------------
            def epi1(t, po):
                xb = wpool.tile([128, FD], F32, tag="epi")
                nc.vector.tensor_tensor(out=xb[:], in0=po[:], in1=t_b1[:],
                                        op=mybir.AluOpType.add)
                nc.scalar.activation(
                    out=t_u2[:, t, :], in_=xb[:],
                    func=mybir.ActivationFunctionType.Relu,
                    scale=t_dinv[:, t:t + 1],
                )

            cc2_view = cc_in2.rearrange("(t p) f -> p t f", p=128)

            def post_band(b):
                nc.sync.dma_start(
                    out=cc2_view[:, BAND_T0[b]:BAND_T1[b], 0:FD],
                    in_=t_u2[:, BAND_T0[b]:BAND_T1[b], :],
                )
                nc.gpsimd.collective_compute(
                    "AllGather", mybir.AluOpType.bypass,
                    ins=[cc_in2[BAND_T0[b] * TS:BAND_T1[b] * TS, :]],
                    outs=[u2full[b]],
                    replica_groups=[list(range(NC_CORES))],
                )

            layer(u1full, t_u1, t_w1, FD, epi1, post_band, prefix=5)

            # ---- layer 2 -------------------------------------------------
            def epi2(t, po):
                ob = wpool.tile([128, OD], F32, tag="epi")
                nc.vector.tensor_tensor(out=ob[:], in0=po[:], in1=t_b2[:],
                                        op=mybir.AluOpType.add)
                nc.sync.dma_start(out=d_out[:, t, :], in_=ob[:])

            layer(u2full, t_u2, t_w2, OD, epi2, lambda b: None,
                  prefix=5)

    nc.compile()
    return nc


# ---------------------------------------------------------------- entry point
def kernel(x, W1, b1, W2, b2, edge_index):
    x = np.asarray(x, dtype=np.float32)
    W1 = np.asarray(W1, dtype=np.float32)
    b1 = np.asarray(b1, dtype=np.float32)
    W2 = np.asarray(W2, dtype=np.float32)
    b2 = np.asarray(b2, dtype=np.float32)
    edge_index = np.asarray(edge_index)

    ekey = hash(edge_index.tobytes())
    if ekey in _compiled_cache:
        nc, meta = _compiled_cache[ekey]
    else:
        meta = _prep_edges(edge_index)
        (runs, run_start, sec_base, sec_len, tot, idx_s, rel_s,
         tile_mms, sec_mm_len, mm_tot_pad) = meta
        nc = _build(runs, run_start, sec_base, sec_len, tot, tile_mms,
                    sec_mm_len, mm_tot_pad)
        _compiled_cache[ekey] = (nc, meta)
    (runs, run_start, sec_base, sec_len, tot, idx_s, rel_s,
     tile_mms, sec_mm_len, mm_tot_pad) = meta

    dst = np.asarray(edge_index[1], dtype=np.int64)
    deg_full = np.bincount(dst, minlength=N_NODES).astype(np.float32) + 1.0

    iota_np = np.tile(np.arange(TS, dtype=np.float16)[None, :], (128, 1))
    id32_np = np.eye(128, dtype=np.float32)
    id16_np = np.eye(128, dtype=np.float16)
    b1rep = np.tile(b1[None, :], (128, 1)).astype(np.float32)
    b2rep = np.tile(b2[None, :], (128, 1)).astype(np.float32)

    in_maps = []
    for k in range(NC_CORES):
        xs = np.zeros((SP, FD), dtype=np.float32)
        xs[:S] = x[k * S:(k + 1) * S]
        degs = np.ones((SP,), dtype=np.float32)
        degs[:S] = deg_full[k * S:(k + 1) * S]
        in_maps.append({
            "x_shard": xs.reshape(TPC, 128, FD).transpose(1, 0, 2).copy(),
            "deg_shard": degs.reshape(TPC, 128).T.copy(),
            "idx_stream": _wrap_idx(idx_s[k]),
            "rel_stream": _wrap_rel(rel_s[k]),
            "iota16": iota_np, "ident32": id32_np, "ident16": id16_np,
            "W1": W1, "b1rep": b1rep, "W2": W2, "b2rep": b2rep,
        })

    trace = bool(os.environ.get("BASS_TRACE"))
    res = run_bass_kernel_spmd(
        nc, in_maps, core_ids=list(range(NC_CORES)), trace=trace,
    )
    if trace and res.exec_time_ns is not None:
        print(f"HW exec time: {res.exec_time_ns} ns")
        kernel.last_exec_time_ns = res.exec_time_ns

    outs = []
    for k in range(NC_CORES):
        o = res.results[k]["out_shard"]          # [128, TPC, OD]
        outs.append(o.transpose(1, 0, 2).reshape(SP, OD)[:S])
    return np.concatenate(outs, axis=0)


# revision 4
# speedup vs baseline: 1.4783x; 1.0121x over previous
"""Bass/Trainium2 kernel for 2-layer GCN (nn_MeshGNN), 8 NeuronCores. v2.

Math (per layer, commuted form):
    A_hat = D^-1/2 (A+I) D^-1/2 ;  gcn(x) = A_hat x W + b
    u = dinv * x ;  agg[d] = sum_{e: dst=d} u[src[e]] + u[d]
    out = (dinv * agg) @ W + b           (layer 1 adds relu)

Distribution: nodes sharded by range across 8 cores (12500/core, padded to
12544 = 98 tiles of 128). The gather table u (fp16, 256B rows) is exchanged
in 4 row-BANDS via per-band AllGathers: band b holds tiles band_t0[b]:band_t1[b]
of EVERY core's shard, so chunk-c gathers unblock as soon as band c's
AllGather lands (layer 2's first AG fires mid-layer-1). The layer sweep is
chunk-major, accumulating per-tile partials in an SBUF f32 accumulator.
Edge streams (gather indices + relative-dst) are identical for both layers.
"""
import os
import numpy as np

import concourse.bacc as bacc
import concourse.mybir as mybir
from concourse.tile import TileContext
from concourse.bass_utils import run_bass_kernel_spmd

# ---------------------------------------------------------------- constants
N_NODES = 100000
NC_CORES = 8
S = 12500                 # nodes per core
TS = 128                  # dst-tile size
TPC = 98                  # dst tiles per core (98*128 = 12544)
SP = TPC * TS             # padded nodes per core
NCH = 4                   # src bands
BAND_TILES = [25, 25, 24, 24]
BAND_T0 = [sum(BAND_TILES[:i]) for i in range(NCH)]
BAND_T1 = [sum(BAND_TILES[:i + 1]) for i in range(NCH)]
CHR = [8 * 128 * bt for bt in BAND_TILES]   # table rows per band chunk
CHB = [sum(CHR[:i]) for i in range(NCH)]    # band chunk base row in big table
FD = 64                   # in/hidden feature dim
OD = 32                   # output dim
BLK = 1024                # gather block (slots per dma_gather, single-packet max)
OHG = 1024                # one-hot group (slots per DVE op)
F32 = mybir.dt.float32
F16 = mybir.dt.float16
I16 = mybir.dt.int16

_compiled_cache = {}


# ---------------------------------------------------------------- tile patch
def _install_tile_patch():
    """walrus here rejects >1 sync-wait on an InstDrain; split the Tile tail
    drain's waits across sequential drains (same engine => same semantics)."""
    from bass_rust import ScopedClock

    def _patched(self, tick_clock, wait_clock):
        drain_inst = self.nc.sync.drain()
        wait_clock.add_sem_waits(
            drain_inst.ins, ScopedClock({None: tick_clock.global_clock})
        )
        si = drain_inst.ins.sync_info
        waits = list(si.on_wait) if si and si.on_wait else []
        if len(waits) > 1:
            si.on_wait = waits[:1]
            for w in waits[1:]:
                extra = self.nc.sync.drain()
                extra.ins.sync_info = mybir.SyncInfo(on_wait=[w], on_update=[])
        self.nc.all_engine_barrier()
        assert self.sems is not None
        popped = self.nc._tile_sem_poison_stack.pop()
        assert popped is self._sem_poison
        self.nc.clear_and_free_semaphores(list(self.sems.allocated().values()))
        self.nc.all_engine_barrier()

    TileContext._drain_and_barrier = _patched


_install_tile_patch()


# ---------------------------------------------------------------- host prep
def _prep_edges(edge_index):
    """Shared-run-structure edge streams, band chunks. Returns (runs_padded
    [NCH,TPC], run_start [NCH,TPC], sec_base [NCH], sec_len [NCH], tot,
    idx_streams [NC][tot], rel_streams [NC][tot])."""
    src = np.asarray(edge_index[0], dtype=np.int64)
    dst = np.asarray(edge_index[1], dtype=np.int64)

    # source table row within its band chunk
    k = src // S
    pos = src % S
    stile = pos // TS
    band_of_tile = np.zeros(TPC, dtype=np.int64)
    for b in range(NCH):
        band_of_tile[BAND_T0[b]:BAND_T1[b]] = b
    band = band_of_tile[stile]
    t0 = np.asarray(BAND_T0, dtype=np.int64)
    trow = k * (np.asarray(BAND_TILES)[band] * TS) \
        + (stile - t0[band]) * TS + pos % TS

    core = dst // S
    dstloc = dst - core * S
    tile = dstloc // TS

    key = (core * NCH + band) * TPC + tile
    counts = np.bincount(key, minlength=NC_CORES * NCH * TPC).reshape(
        NC_CORES, NCH, TPC
    )
    runs = counts.max(axis=0)                       # [NCH, TPC] exact max
    sec_len = runs.sum(axis=1)
    # pad each section to BLK multiple so gather blocks stay in-section
    sec_len = ((sec_len + BLK - 1) // BLK) * BLK
    sec_base = np.concatenate([[0], np.cumsum(sec_len)[:-1]])
    run_start = sec_base[:, None] + (np.cumsum(runs, axis=1) - runs)
    tot = int(sec_len.sum())

    # per-(c,t) matmul list: columns of 128 slots overlapping the run;
    # per-mm one-hot source = rel values masked to the run's slot range.
    tile_mms = [[None] * TPC for _ in range(NCH)]
    sec_mm_len = np.zeros(NCH, dtype=np.int64)
    mm_slot = []       # global mm -> (slot_lo, slot_hi, col_base)
    for c in range(NCH):
        m0 = 0
        for t in range(TPC):
            a = int(run_start[c, t]) - int(sec_base[c])
            b = a + int(runs[c, t])
            cols = range(a // TS, (b - 1) // TS + 1) if b > a else []
            tile_mms[c][t] = [(m0 + i, j) for i, j in enumerate(cols)]
            for j in cols:
                mm_slot.append((c, a, b, j * TS))
            m0 += len(tile_mms[c][t])
        sec_mm_len[c] = m0
    mm_tot = int(sec_mm_len.sum())
    mm_tot_pad = ((mm_tot + 7) // 8) * 8

    idx_streams, rel_streams = [], []
    for kk in range(NC_CORES):
        sel = core == kk
        c_k, t_k = band[sel], tile[sel]
        row_k = trow[sel]
        rel_k = (dstloc[sel] - t_k * TS).astype(np.float16)
        # within each (band, tile) run, order slots by ascending table row:
        # the gather's random reads become monotonic (DRAM-friendlier)
        order = np.lexsort((row_k, t_k, c_k))
        c_k, t_k, row_k, rel_k = c_k[order], t_k[order], row_k[order], rel_k[order]
        key_k = c_k * TPC + t_k
        cnt_k = np.bincount(key_k, minlength=NCH * TPC)
        grp_start = np.cumsum(cnt_k) - cnt_k
        within = np.arange(len(key_k)) - grp_start[key_k]
        slot = run_start.reshape(-1)[key_k] + within
        idx_s = np.zeros(tot, dtype=np.int16)
        rel_s = np.full(tot, -1.0, dtype=np.float16)
        idx_s[slot] = row_k.astype(np.int16)
        rel_s[slot] = rel_k
        # per-mm masked rel stream [mm_tot_pad * 128]
        relmm = np.full(mm_tot_pad * TS, -1.0, dtype=np.float16)
        for m, (c, a, b, col0) in enumerate(mm_slot):
            lo = max(a, col0)
            hi = min(b, col0 + TS)
            src = rel_s[int(sec_base[c]) + lo:int(sec_base[c]) + hi]
            relmm[m * TS + (lo - col0):m * TS + (hi - col0)] = src
        idx_streams.append(idx_s)
        rel_streams.append(relmm)
    return (runs, run_start, sec_base, sec_len, tot, idx_streams, rel_streams,
            tile_mms, sec_mm_len, mm_tot_pad)


def _wrap_idx(idx_s):
    """[tot] int16 -> [128, tot/16] wrapped + replicated across 8 groups."""
    tot = idx_s.shape[0]
    w = idx_s.reshape(tot // 16, 16).T              # [16, tot/16]
    return np.tile(w, (8, 1)).copy()                # [128, tot/16]


def _wrap_rel(rel_s):
    tot = rel_s.shape[0]
    return rel_s.reshape(tot // 128, 128).T.copy()  # [128, tot/128]


# ---------------------------------------------------------------- kernel build
def _build(runs, run_start, sec_base, sec_len, tot, tile_mms, sec_mm_len,
           mm_tot_pad):
    sec_mm_base = np.concatenate([[0], np.cumsum(sec_mm_len)[:-1]])
    nc = bacc.Bacc(None, target_bir_lowering=False, debug=False,
                   num_devices=NC_CORES, num_swdge_queues=4)

    # ---- I/O -------------------------------------------------------------
    d_u1f = nc.dram_tensor("u1full_in", [SP * NC_CORES, 128], F16,
                           kind="ExternalInput")
    d_u1s = nc.dram_tensor("u1self", [128, TPC, FD], F16, kind="ExternalInput")
    d_deg = nc.dram_tensor("deg_shard", [128, TPC], F32, kind="ExternalInput")
    d_idx = nc.dram_tensor("idx_stream", [128, tot // 16], I16, kind="ExternalInput")
    d_rel = nc.dram_tensor("rel_stream", [128, mm_tot_pad], F16, kind="ExternalInput")
    d_iota = nc.dram_tensor("iota16", [128, TS], F16, kind="ExternalInput")
    d_id32 = nc.dram_tensor("ident32", [128, 128], F32, kind="ExternalInput")
    d_id16 = nc.dram_tensor("ident16", [128, 128], F16, kind="ExternalInput")
    d_w1 = nc.dram_tensor("W1", [FD, FD], F32, kind="ExternalInput")
    d_b1 = nc.dram_tensor("b1rep", [128, FD], F32, kind="ExternalInput")
    d_w2 = nc.dram_tensor("W2", [FD, OD], F32, kind="ExternalInput")
    d_b2 = nc.dram_tensor("b2rep", [128, OD], F32, kind="ExternalInput")
    d_out = nc.dram_tensor("out_shard", [128, TPC, OD], F32, kind="ExternalOutput")

    cc_in2 = nc.dram_tensor("cc_in2", [SP, 128], F16, kind="Internal")
    u2big = nc.dram_tensor("u2full", [SP * NC_CORES, 128], F16,
                           kind="Internal", addr_space="Shared")
    u1full = [d_u1f[CHB[c]:CHB[c] + CHR[c], :] for c in range(NCH)]
    u2full = [u2big[CHB[c]:CHB[c] + CHR[c], :] for c in range(NCH)]

    with TileContext(nc) as tc:
        with (
            tc.tile_pool(name="const", bufs=1) as cpool,
            tc.tile_pool(name="stage", bufs=1) as spool,
            tc.tile_pool(name="msg", bufs=7) as mpool,
            tc.tile_pool(name="oh", bufs=3) as opool,
            tc.tile_pool(name="work", bufs=4) as wpool,
            tc.tile_pool(name="psA", bufs=4, space="PSUM") as psA,
            tc.tile_pool(name="psB", bufs=2, space="PSUM") as psB,
            tc.tile_pool(name="psC", bufs=2, space="PSUM") as psC,
        ):
            # ---- constants / streams ------------------------------------
            t_idx = cpool.tile([128, tot // 16], I16)
            nc.sync.dma_start(out=t_idx[:], in_=d_idx[:, :])
            t_rel = cpool.tile([128, mm_tot_pad], F16)
            nc.sync.dma_start(out=t_rel[:], in_=d_rel[:, :])
            t_iota = cpool.tile([128, TS], F16)
            nc.sync.dma_start(out=t_iota[:], in_=d_iota[:, :])
            t_id32 = cpool.tile([128, 128], F32)
            nc.sync.dma_start(out=t_id32[:], in_=d_id32[:, :])
            t_id16 = cpool.tile([128, 128], F16)
            nc.sync.dma_start(out=t_id16[:], in_=d_id16[:, :])
            t_w1 = cpool.tile([FD, FD], F32)
            nc.sync.dma_start(out=t_w1[:], in_=d_w1[:, :])
            t_b1 = cpool.tile([128, FD], F32)
            nc.sync.dma_start(out=t_b1[:], in_=d_b1[:, :])
            t_w2 = cpool.tile([FD, OD], F32)
            nc.sync.dma_start(out=t_w2[:], in_=d_w2[:, :])
            t_b2 = cpool.tile([128, OD], F32)
            nc.sync.dma_start(out=t_b2[:], in_=d_b2[:, :])

            # ---- dinv ----------------------------------------------------
            t_deg = cpool.tile([128, TPC], F32)
            nc.sync.dma_start(out=t_deg[:], in_=d_deg[:, :])
            t_dinv = cpool.tile([128, TPC], F32)
            nc.vector.reciprocal(out=t_dinv[:], in_=t_deg[:])
            nc.scalar.activation(out=t_dinv[:], in_=t_dinv[:],
                                 func=mybir.ActivationFunctionType.Sqrt)

            # ---- u1 table is host-precomputed (pure input function) ------
            t_u1 = spool.tile([128, TPC, FD], F16)
            nc.sync.dma_start(out=t_u1[:], in_=d_u1s[:, :, :])

            # ---- one shared layer (tile-major sweep, band AG overlap) ----
            t_u2 = spool.tile([128, TPC, FD], F16)

            def layer(ufull, u_stage, w_tile, outd, epilogue, post_band,
                      prefix=0):
                msg_tiles = {}
                oh_tiles = {}
                cursor_blk = [0] * NCH
                cursor_ohg = [0] * NCH

                def ensure(c, upto_slot, upto_mm):
                    while cursor_blk[c] * BLK < upto_slot:
                        bi = cursor_blk[c]
                        ln = min(BLK, int(sec_len[c]) - bi * BLK)
                        blk = mpool.tile([128, BLK // 128, 128], F16,
                                         tag=f"msg{c}")
                        a = int(sec_base[c]) + bi * BLK
                        nc.gpsimd.dma_gather(
                            blk[:, 0:ln // 128, :],
                            ufull[c],
                            t_idx[:, a // 16:(a + ln) // 16],
                            ln, ln, 128,
                            single_packet=True,
                            queue_num=c % 4,
                        )
                        msg_tiles[(c, bi)] = blk
                        cursor_blk[c] = bi + 1
                    while cursor_ohg[c] * 8 < upto_mm:
                        gi = cursor_ohg[c]
                        nb = min(8, int(sec_mm_len[c]) - gi * 8)
                        ohp = opool.tile([128, 8, TS], F16, tag=f"oh{c}")
                        g0 = int(sec_mm_base[c]) + gi * 8
                        nc.vector.tensor_tensor(
                            out=ohp[:, 0:nb, :],
                            in0=t_rel[:, g0:g0 + nb, None].to_broadcast(
                                [128, nb, TS]),
                            in1=t_iota[:, None, :].to_broadcast([128, nb, TS]),
                            op=mybir.AluOpType.is_equal,
                        )
                        oh_tiles[(c, gi)] = ohp
                        cursor_ohg[c] = gi + 1

                if prefix:
                    for c in range(NCH - 1):
                        ensure(c, prefix * BLK, prefix * 8)
                for t in range(TPC):
                    for c in range(NCH):
                        rs = int(run_start[c, t] - sec_base[c])
                        rl = int(runs[c, t])
                        if tile_mms[c][t]:
                            ensure(c, rs + rl, tile_mms[c][t][-1][0] + 1)
                    ps = psA.tile([128, FD], F32, tag="agg")
                    nc.tensor.matmul(out=ps[:], lhsT=t_id16[:],
                                     rhs=u_stage[:, t, :],
                                     start=True, stop=False)
                    mms = []
                    for c in range(NCH):
                        for m, j in tile_mms[c][t]:
                            mms.append((c, m, j))
                    for i, (c, m, j) in enumerate(mms):
                        oh = oh_tiles[(c, m // 8)]
                        mg = msg_tiles[(c, j * TS // BLK)]
                        nc.tensor.matmul(
                            out=ps[:],
                            lhsT=oh[:, m % 8, :],
                            rhs=mg[:, (j * TS % BLK) // 128, 0:FD],
                            start=False, stop=(i == len(mms) - 1),
                        )
                    assert mms
                    # epilogue: pre = dinv * agg; preT; po = preT.T @ W
                    pre = wpool.tile([128, FD], F32, tag="pre")
                    nc.vector.tensor_scalar(
                        out=pre[:], in0=ps[:], scalar1=t_dinv[:, t:t + 1],
                        scalar2=None, op0=mybir.AluOpType.mult,
                    )
                    pst = psB.tile([FD, 128], F32, tag="tr")
                    nc.tensor.transpose(out=pst[:], in_=pre[:],
                                        identity=t_id32[:])
                    preT = wpool.tile([FD, 128], F32, tag="preT")
                    nc.scalar.copy(out=preT[:], in_=pst[:])
                    po = psC.tile([128, outd], F32, tag="mm2")
                    nc.tensor.matmul(out=po[:], lhsT=preT[:], rhs=w_tile[:],
                                     start=True, stop=True)
                    epilogue(t, po)
                    for b in range(NCH):
                        if t == BAND_T1[b] - 1:
                            post_band(b)

            # ---- layer 1 -------------------------------------------------
            def epi1(t, po):
                xb = wpool.tile([128, FD], F32, tag="epi")
                nc.vector.tensor_tensor(out=xb[:], in0=po[:], in1=t_b1[:],
                                        op=mybir.AluOpType.add)
                nc.scalar.activation(
                    out=t_u2[:, t, :], in_=xb[:],
                    func=mybir.ActivationFunctionType.Relu,
                    scale=t_dinv[:, t:t + 1],
                )

            cc2_view = cc_in2.rearrange("(t p) f -> p t f", p=128)

            def post_band(b):
                nc.sync.dma_start(
                    out=cc2_view[:, BAND_T0[b]:BAND_T1[b], 0:FD],
                    in_=t_u2[:, BAND_T0[b]:BAND_T1[b], :],
                )
                nc.gpsimd.collective_compute(
                    "AllGather", mybir.AluOpType.bypass,
                    ins=[cc_in2[BAND_T0[b] * TS:BAND_T1[b] * TS, :]],
                    outs=[u2full[b]],
                    replica_groups=[list(range(NC_CORES))],
                )

            layer(u1full, t_u1, t_w1, FD, epi1, post_band, prefix=5)

            # ---- layer 2 -------------------------------------------------
            def epi2(t, po):
                ob = wpool.tile([128, OD], F32, tag="epi")
                nc.vector.tensor_tensor(out=ob[:], in0=po[:], in1=t_b2[:],
                                        op=mybir.AluOpType.add)
                nc.sync.dma_start(out=d_out[:, t, :], in_=ob[:])

            layer(u2full, t_u2, t_w2, OD, epi2, lambda b: None,
                  prefix=5)

    nc.compile()
    return nc


# ---------------------------------------------------------------- entry point
def kernel(x, W1, b1, W2, b2, edge_index):
    x = np.asarray(x, dtype=np.float32)
    W1 = np.asarray(W1, dtype=np.float32)
    b1 = np.asarray(b1, dtype=np.float32)
    W2 = np.asarray(W2, dtype=np.float32)
    b2 = np.asarray(b2, dtype=np.float32)
    edge_index = np.asarray(edge_index)

    ekey = hash(edge_index.tobytes())
    if ekey in _compiled_cache:
        nc, meta = _compiled_cache[ekey]
    else:
        meta = _prep_edges(edge_index)
        (runs, run_start, sec_base, sec_len, tot, idx_s, rel_s,
         tile_mms, sec_mm_len, mm_tot_pad) = meta
        nc = _build(runs, run_start, sec_base, sec_len, tot, tile_mms,
                    sec_mm_len, mm_tot_pad)
        _compiled_cache[ekey] = (nc, meta)
    (runs, run_start, sec_base, sec_len, tot, idx_s, rel_s,
     tile_mms, sec_mm_len, mm_tot_pad) = meta

    dst = np.asarray(edge_index[1], dtype=np.int64)
    deg_full = np.bincount(dst, minlength=N_NODES).astype(np.float32) + 1.0

    iota_np = np.tile(np.arange(TS, dtype=np.float16)[None, :], (128, 1))
    id32_np = np.eye(128, dtype=np.float32)
    id16_np = np.eye(128, dtype=np.float16)
    b1rep = np.tile(b1[None, :], (128, 1)).astype(np.float32)
    b2rep = np.tile(b2[None, :], (128, 1)).astype(np.float32)

    # host-precomputed u1 = dinv * x, padded per core, band-major table
    dinv_full = 1.0 / np.sqrt(deg_full)
    u1_pad = np.zeros((NC_CORES, SP, FD), dtype=np.float16)
    for k in range(NC_CORES):
        u1_pad[k, :S] = (x[k * S:(k + 1) * S]
                         * dinv_full[k * S:(k + 1) * S, None])
    u1full_host = np.zeros((SP * NC_CORES, 128), dtype=np.float16)
    for c in range(NC_CORES if False else len(CHB)):
        bt0, bt1 = BAND_T0[c] * TS, BAND_T1[c] * TS
        nrow = bt1 - bt0
        for k in range(NC_CORES):
            r0 = CHB[c] + k * nrow
            u1full_host[r0:r0 + nrow, 0:FD] = u1_pad[k, bt0:bt1]

    in_maps = []
    for k in range(NC_CORES):
        degs = np.ones((SP,), dtype=np.float32)
        degs[:S] = deg_full[k * S:(k + 1) * S]
        in_maps.append({
            "u1full_in": u1full_host,
            "u1self": u1_pad[k].reshape(TPC, 128, FD).transpose(1, 0, 2).copy(),
            "deg_shard": degs.reshape(TPC, 128).T.copy(),
            "idx_stream": _wrap_idx(idx_s[k]),
            "rel_stream": _wrap_rel(rel_s[k]),
            "iota16": iota_np, "ident32": id32_np, "ident16": id16_np,
            "W1": W1, "b1rep": b1rep, "W2": W2, "b2rep": b2rep,
        })

    trace = bool(os.environ.get("BASS_TRACE"))
    res = run_bass_kernel_spmd(
        nc, in_maps, core_ids=list(range(NC_CORES)), trace=trace,
    )
    if trace and res.exec_time_ns is not None:
        print(f"HW exec time: {res.exec_time_ns} ns")
        kernel.last_exec_time_ns = res.exec_time_ns

    outs = []
    for k in range(NC_CORES):
        o = res.results[k]["out_shard"]          # [128, TPC, OD]
        outs.append(o.transpose(1, 0, 2).reshape(SP, OD)[:S])
    return np.concatenate(outs, axis=0)


# revision 5
# speedup vs baseline: 1.5390x; 1.0411x over previous
"""Bass/Trainium2 kernel for 2-layer GCN (nn_MeshGNN), 8 NeuronCores. v2.

Math (per layer, commuted form):
    A_hat = D^-1/2 (A+I) D^-1/2 ;  gcn(x) = A_hat x W + b
    u = dinv * x ;  agg[d] = sum_{e: dst=d} u[src[e]] + u[d]
    out = (dinv * agg) @ W + b           (layer 1 adds relu)

Distribution: nodes sharded by range across 8 cores (12500/core, padded to
12544 = 98 tiles of 128). The gather table u (fp16, 256B rows) is exchanged
in 4 row-BANDS via per-band AllGathers: band b holds tiles band_t0[b]:band_t1[b]
of EVERY core's shard, so chunk-c gathers unblock as soon as band c's
AllGather lands (layer 2's first AG fires mid-layer-1). The layer sweep is
chunk-major, accumulating per-tile partials in an SBUF f32 accumulator.
Edge streams (gather indices + relative-dst) are identical for both layers.
"""
import os
import numpy as np

import concourse.bacc as bacc
import concourse.mybir as mybir
from concourse.tile import TileContext
from concourse.bass_utils import run_bass_kernel_spmd

# ---------------------------------------------------------------- constants
N_NODES = 100000
NC_CORES = 8
S = 12500                 # nodes per core
TS = 128                  # dst-tile size
TPC = 98                  # dst tiles per core (98*128 = 12544)
SP = TPC * TS             # padded nodes per core
NCH = 4                   # src bands
BAND_TILES = [25, 25, 24, 24]
BAND_T0 = [sum(BAND_TILES[:i]) for i in range(NCH)]
BAND_T1 = [sum(BAND_TILES[:i + 1]) for i in range(NCH)]
CHR = [8 * 128 * bt for bt in BAND_TILES]   # table rows per band chunk
CHB = [sum(CHR[:i]) for i in range(NCH)]    # band chunk base row in big table
FD = 64                   # in/hidden feature dim
OD = 32                   # output dim
BLK = 1024                # gather block (slots per dma_gather, single-packet max)
OHG = 1024                # one-hot group (slots per DVE op)
F32 = mybir.dt.float32
F16 = mybir.dt.float16
I16 = mybir.dt.int16

_compiled_cache = {}


# ---------------------------------------------------------------- tile patch
def _install_tile_patch():
    """walrus here rejects >1 sync-wait on an InstDrain; split the Tile tail
    drain's waits across sequential drains (same engine => same semantics)."""
    from bass_rust import ScopedClock

    def _patched(self, tick_clock, wait_clock):
        drain_inst = self.nc.sync.drain()
        wait_clock.add_sem_waits(
            drain_inst.ins, ScopedClock({None: tick_clock.global_clock})
        )
        si = drain_inst.ins.sync_info
        waits = list(si.on_wait) if si and si.on_wait else []
        if len(waits) > 1:
            si.on_wait = waits[:1]
            for w in waits[1:]:
                extra = self.nc.sync.drain()
                extra.ins.sync_info = mybir.SyncInfo(on_wait=[w], on_update=[])
        self.nc.all_engine_barrier()
        assert self.sems is not None
        popped = self.nc._tile_sem_poison_stack.pop()
        assert popped is self._sem_poison
        self.nc.clear_and_free_semaphores(list(self.sems.allocated().values()))
        self.nc.all_engine_barrier()

    TileContext._drain_and_barrier = _patched


_install_tile_patch()


# ---------------------------------------------------------------- host prep
def _compute_pos_map(src, dst):
    """Within-core padded positions balancing per-(band,tile) in-degree.
    Nodes keep their band (so source-band profiles stay exact) but are dealt
    greedily among the band's tiles to flatten max-over-cores run lengths."""
    k_dst = dst // S
    pos0 = np.arange(N_NODES, dtype=np.int64) % S
    band_of_tile = np.zeros(TPC, dtype=np.int64)
    for b in range(NCH):
        band_of_tile[BAND_T0[b]:BAND_T1[b]] = b
    band_src = band_of_tile[(np.arange(N_NODES) % S) // TS]
    # per-node per-band in-degree profile
    prof = np.bincount(dst * NCH + band_src[src],
                       minlength=N_NODES * NCH).reshape(N_NODES, NCH)
    pos = np.zeros(N_NODES, dtype=np.int64)
    for k in range(NC_CORES):
        nodes_k = np.arange(k * S, (k + 1) * S)
        for b in range(NCH):
            t0, t1 = BAND_T0[b], BAND_T1[b]
            nt = t1 - t0
            sel = nodes_k[(pos0[nodes_k] // TS >= t0)
                          & (pos0[nodes_k] // TS < t1)]
            p = prof[sel].astype(np.float64)          # [n, NCH]
            order = np.argsort(-p.sum(axis=1))
            load = np.zeros((nt, NCH))
            cap = np.full(nt, TS, dtype=np.int64)
            cap[:] = TS
            slots_used = np.zeros(nt, dtype=np.int64)
            target = p.sum(axis=0) / nt
            for n in sel[order]:
                cand = np.where(slots_used < TS)[0]
                score = (load[cand] + prof[n][None, :] - target[None, :]
                         ).max(axis=1)
                j = cand[np.argmin(score)]
                load[j] += prof[n]
                pos[n] = (t0 + j) * TS + slots_used[j]
                slots_used[j] += 1
    return pos


def _prep_edges(edge_index):
    """Shared-run-structure edge streams, band chunks. Returns (runs_padded
    [NCH,TPC], run_start [NCH,TPC], sec_base [NCH], sec_len [NCH], tot,
    idx_streams [NC][tot], rel_streams [NC][tot])."""
    src = np.asarray(edge_index[0], dtype=np.int64)
    dst = np.asarray(edge_index[1], dtype=np.int64)
    posmap = _compute_pos_map(src, dst)

    # source table row within its band chunk
    k = src // S
    pos = posmap[src]
    stile = pos // TS
    band_of_tile = np.zeros(TPC, dtype=np.int64)
    for b in range(NCH):
        band_of_tile[BAND_T0[b]:BAND_T1[b]] = b
    band = band_of_tile[stile]
    t0 = np.asarray(BAND_T0, dtype=np.int64)
    trow = k * (np.asarray(BAND_TILES)[band] * TS) \
        + (stile - t0[band]) * TS + pos % TS

    core = dst // S
    dstloc = posmap[dst]
    tile = dstloc // TS

    key = (core * NCH + band) * TPC + tile
    counts = np.bincount(key, minlength=NC_CORES * NCH * TPC).reshape(
        NC_CORES, NCH, TPC
    )
    runs = counts.max(axis=0)                       # [NCH, TPC] exact max
    sec_len = runs.sum(axis=1)
    # pad each section to BLK multiple so gather blocks stay in-section
    sec_len = ((sec_len + BLK - 1) // BLK) * BLK
    sec_base = np.concatenate([[0], np.cumsum(sec_len)[:-1]])
    run_start = sec_base[:, None] + (np.cumsum(runs, axis=1) - runs)
    tot = int(sec_len.sum())

    # per-(c,t) matmul list: columns of 128 slots overlapping the run;
    # per-mm one-hot source = rel values masked to the run's slot range.
    tile_mms = [[None] * TPC for _ in range(NCH)]
    sec_mm_len = np.zeros(NCH, dtype=np.int64)
    mm_slot = []       # global mm -> (slot_lo, slot_hi, col_base)
    for c in range(NCH):
        m0 = 0
        for t in range(TPC):
            a = int(run_start[c, t]) - int(sec_base[c])
            b = a + int(runs[c, t])
            cols = range(a // TS, (b - 1) // TS + 1) if b > a else []
            tile_mms[c][t] = [(m0 + i, j) for i, j in enumerate(cols)]
            for j in cols:
                mm_slot.append((c, a, b, j * TS))
            m0 += len(tile_mms[c][t])
        sec_mm_len[c] = m0
    mm_tot = int(sec_mm_len.sum())
    mm_tot_pad = ((mm_tot + 7) // 8) * 8

    idx_streams, rel_streams = [], []
    for kk in range(NC_CORES):
        sel = core == kk
        c_k, t_k = band[sel], tile[sel]
        row_k = trow[sel]
        rel_k = (dstloc[sel] - t_k * TS).astype(np.float16)
        # within each (band, tile) run, order slots by ascending table row:
        # the gather's random reads become monotonic (DRAM-friendlier)
        order = np.lexsort((row_k, t_k, c_k))
        c_k, t_k, row_k, rel_k = c_k[order], t_k[order], row_k[order], rel_k[order]
        key_k = c_k * TPC + t_k
        cnt_k = np.bincount(key_k, minlength=NCH * TPC)
        grp_start = np.cumsum(cnt_k) - cnt_k
        within = np.arange(len(key_k)) - grp_start[key_k]
        slot = run_start.reshape(-1)[key_k] + within
        idx_s = np.zeros(tot, dtype=np.int16)
        rel_s = np.full(tot, -1.0, dtype=np.float16)
        idx_s[slot] = row_k.astype(np.int16)
        rel_s[slot] = rel_k
        # per-mm masked rel stream [mm_tot_pad * 128]
        relmm = np.full(mm_tot_pad * TS, -1.0, dtype=np.float16)
        for m, (c, a, b, col0) in enumerate(mm_slot):
            lo = max(a, col0)
            hi = min(b, col0 + TS)
            src = rel_s[int(sec_base[c]) + lo:int(sec_base[c]) + hi]
            relmm[m * TS + (lo - col0):m * TS + (hi - col0)] = src
        idx_streams.append(idx_s)
        rel_streams.append(relmm)
    return (runs, run_start, sec_base, sec_len, tot, idx_streams, rel_streams,
            tile_mms, sec_mm_len, mm_tot_pad, posmap)


def _wrap_idx(idx_s):
    """[tot] int16 -> [128, tot/16] wrapped + replicated across 8 groups."""
    tot = idx_s.shape[0]
    w = idx_s.reshape(tot // 16, 16).T              # [16, tot/16]
    return np.tile(w, (8, 1)).copy()                # [128, tot/16]


def _wrap_rel(rel_s):
    tot = rel_s.shape[0]
    return rel_s.reshape(tot // 128, 128).T.copy()  # [128, tot/128]


# ---------------------------------------------------------------- kernel build
def _build(runs, run_start, sec_base, sec_len, tot, tile_mms, sec_mm_len,
           mm_tot_pad):
    sec_mm_base = np.concatenate([[0], np.cumsum(sec_mm_len)[:-1]])
    nc = bacc.Bacc(None, target_bir_lowering=False, debug=False,
                   num_devices=NC_CORES, num_swdge_queues=4)

    # ---- I/O -------------------------------------------------------------
    d_u1f = nc.dram_tensor("u1full_in", [SP * NC_CORES, 128], F16,
                           kind="ExternalInput")
    d_u1s = nc.dram_tensor("u1self", [128, TPC, FD], F16, kind="ExternalInput")
    d_deg = nc.dram_tensor("deg_shard", [128, TPC], F32, kind="ExternalInput")
    d_idx = nc.dram_tensor("idx_stream", [128, tot // 16], I16, kind="ExternalInput")
    d_rel = nc.dram_tensor("rel_stream", [128, mm_tot_pad], F16, kind="ExternalInput")
    d_iota = nc.dram_tensor("iota16", [128, TS], F16, kind="ExternalInput")
    d_id32 = nc.dram_tensor("ident32", [128, 128], F32, kind="ExternalInput")
    d_id16 = nc.dram_tensor("ident16", [128, 128], F16, kind="ExternalInput")
    d_w1 = nc.dram_tensor("W1", [FD, FD], F32, kind="ExternalInput")
    d_b1 = nc.dram_tensor("b1rep", [128, FD], F32, kind="ExternalInput")
    d_w2 = nc.dram_tensor("W2", [FD, OD], F32, kind="ExternalInput")
    d_b2 = nc.dram_tensor("b2rep", [128, OD], F32, kind="ExternalInput")
    d_out = nc.dram_tensor("out_shard", [128, TPC, OD], F32, kind="ExternalOutput")

    cc_in2 = nc.dram_tensor("cc_in2", [SP, 128], F16, kind="Internal")
    u2big = nc.dram_tensor("u2full", [SP * NC_CORES, 128], F16,
                           kind="Internal", addr_space="Shared")
    u1full = [d_u1f[CHB[c]:CHB[c] + CHR[c], :] for c in range(NCH)]
    u2full = [u2big[CHB[c]:CHB[c] + CHR[c], :] for c in range(NCH)]

    with TileContext(nc) as tc:
        with (
            tc.tile_pool(name="const", bufs=1) as cpool,
            tc.tile_pool(name="stage", bufs=1) as spool,
            tc.tile_pool(name="msg", bufs=7) as mpool,
            tc.tile_pool(name="oh", bufs=3) as opool,
            tc.tile_pool(name="work", bufs=4) as wpool,
            tc.tile_pool(name="psA", bufs=4, space="PSUM") as psA,
            tc.tile_pool(name="psB", bufs=2, space="PSUM") as psB,
            tc.tile_pool(name="psC", bufs=2, space="PSUM") as psC,
        ):
            # ---- constants / streams ------------------------------------
            t_idx = cpool.tile([128, tot // 16], I16)
            nc.sync.dma_start(out=t_idx[:], in_=d_idx[:, :])
            t_rel = cpool.tile([128, mm_tot_pad], F16)
            nc.sync.dma_start(out=t_rel[:], in_=d_rel[:, :])
            t_iota = cpool.tile([128, TS], F16)
            nc.sync.dma_start(out=t_iota[:], in_=d_iota[:, :])
            t_id32 = cpool.tile([128, 128], F32)
            nc.sync.dma_start(out=t_id32[:], in_=d_id32[:, :])
            t_id16 = cpool.tile([128, 128], F16)
            nc.sync.dma_start(out=t_id16[:], in_=d_id16[:, :])
            t_w1 = cpool.tile([FD, FD], F32)
            nc.sync.dma_start(out=t_w1[:], in_=d_w1[:, :])
            t_b1 = cpool.tile([128, FD], F32)
            nc.sync.dma_start(out=t_b1[:], in_=d_b1[:, :])
            t_w2 = cpool.tile([FD, OD], F32)
            nc.sync.dma_start(out=t_w2[:], in_=d_w2[:, :])
            t_b2 = cpool.tile([128, OD], F32)
            nc.sync.dma_start(out=t_b2[:], in_=d_b2[:, :])

            # ---- dinv ----------------------------------------------------
            t_deg = cpool.tile([128, TPC], F32)
            nc.sync.dma_start(out=t_deg[:], in_=d_deg[:, :])
            t_dinv = cpool.tile([128, TPC], F32)
            nc.vector.reciprocal(out=t_dinv[:], in_=t_deg[:])
            nc.scalar.activation(out=t_dinv[:], in_=t_dinv[:],
                                 func=mybir.ActivationFunctionType.Sqrt)

            # ---- u1 table is host-precomputed (pure input function) ------
            t_u1 = spool.tile([128, TPC, FD], F16)
            nc.sync.dma_start(out=t_u1[:], in_=d_u1s[:, :, :])

            # ---- one shared layer (tile-major sweep, band AG overlap) ----
            t_u2 = spool.tile([128, TPC, FD], F16)

            def layer(ufull, u_stage, w_tile, outd, epilogue, post_band,
                      prefix=0):
                msg_tiles = {}
                oh_tiles = {}
                cursor_blk = [0] * NCH
                cursor_ohg = [0] * NCH

                def ensure(c, upto_slot, upto_mm):
                    while cursor_blk[c] * BLK < upto_slot:
                        bi = cursor_blk[c]
                        ln = min(BLK, int(sec_len[c]) - bi * BLK)
                        blk = mpool.tile([128, BLK // 128, 128], F16,
                                         tag=f"msg{c}")
                        a = int(sec_base[c]) + bi * BLK
                        nc.gpsimd.dma_gather(
                            blk[:, 0:ln // 128, :],
                            ufull[c],
                            t_idx[:, a // 16:(a + ln) // 16],
                            ln, ln, 128,
                            single_packet=True,
                            queue_num=c % 4,
                        )
                        msg_tiles[(c, bi)] = blk
                        cursor_blk[c] = bi + 1
                    while cursor_ohg[c] * 8 < upto_mm:
                        gi = cursor_ohg[c]
                        nb = min(8, int(sec_mm_len[c]) - gi * 8)
                        ohp = opool.tile([128, 8, TS], F16, tag=f"oh{c}")
                        g0 = int(sec_mm_base[c]) + gi * 8
                        nc.vector.tensor_tensor(
                            out=ohp[:, 0:nb, :],
                            in0=t_rel[:, g0:g0 + nb, None].to_broadcast(
                                [128, nb, TS]),
                            in1=t_iota[:, None, :].to_broadcast([128, nb, TS]),
                            op=mybir.AluOpType.is_equal,
                        )
                        oh_tiles[(c, gi)] = ohp
                        cursor_ohg[c] = gi + 1

                if prefix:
                    for c in range(NCH - 1):
                        ensure(c, prefix * BLK, prefix * 8)
                for t in range(TPC):
                    for c in range(NCH):
                        rs = int(run_start[c, t] - sec_base[c])
                        rl = int(runs[c, t])
                        if tile_mms[c][t]:
                            ensure(c, rs + rl, tile_mms[c][t][-1][0] + 1)
                    ps = psA.tile([128, FD], F32, tag="agg")
                    nc.tensor.matmul(out=ps[:], lhsT=t_id16[:],
                                     rhs=u_stage[:, t, :],
                                     start=True, stop=False)
                    mms = []
                    for c in range(NCH):
                        for m, j in tile_mms[c][t]:
                            mms.append((c, m, j))
                    for i, (c, m, j) in enumerate(mms):
                        oh = oh_tiles[(c, m // 8)]
                        mg = msg_tiles[(c, j * TS // BLK)]
                        nc.tensor.matmul(
                            out=ps[:],
                            lhsT=oh[:, m % 8, :],
                            rhs=mg[:, (j * TS % BLK) // 128, 0:FD],
                            start=False, stop=(i == len(mms) - 1),
                        )
                    assert mms
                    # epilogue: pre = dinv * agg; preT; po = preT.T @ W
                    pre = wpool.tile([128, FD], F32, tag="pre")
                    nc.vector.tensor_scalar(
                        out=pre[:], in0=ps[:], scalar1=t_dinv[:, t:t + 1],
                        scalar2=None, op0=mybir.AluOpType.mult,
                    )
                    pst = psB.tile([FD, 128], F32, tag="tr")
                    nc.tensor.transpose(out=pst[:], in_=pre[:],
                                        identity=t_id32[:])
                    preT = wpool.tile([FD, 128], F32, tag="preT")
                    nc.scalar.copy(out=preT[:], in_=pst[:])
                    po = psC.tile([128, outd], F32, tag="mm2")
                    nc.tensor.matmul(out=po[:], lhsT=preT[:], rhs=w_tile[:],
                                     start=True, stop=True)
                    epilogue(t, po)
                    for b in range(NCH):
                        if t == BAND_T1[b] - 1:
                            post_band(b)

            # ---- layer 1 -------------------------------------------------
            def epi1(t, po):
                xb = wpool.tile([128, FD], F32, tag="epi")
                nc.vector.tensor_tensor(out=xb[:], in0=po[:], in1=t_b1[:],
                                        op=mybir.AluOpType.add)
                nc.scalar.activation(
                    out=t_u2[:, t, :], in_=xb[:],
                    func=mybir.ActivationFunctionType.Relu,
                    scale=t_dinv[:, t:t + 1],
                )

            cc2_view = cc_in2.rearrange("(t p) f -> p t f", p=128)

            def post_band(b):
                nc.sync.dma_start(
                    out=cc2_view[:, BAND_T0[b]:BAND_T1[b], 0:FD],
                    in_=t_u2[:, BAND_T0[b]:BAND_T1[b], :],
                )
                nc.gpsimd.collective_compute(
                    "AllGather", mybir.AluOpType.bypass,
                    ins=[cc_in2[BAND_T0[b] * TS:BAND_T1[b] * TS, :]],
                    outs=[u2full[b]],
                    replica_groups=[list(range(NC_CORES))],
                )

            layer(u1full, t_u1, t_w1, FD, epi1, post_band, prefix=5)

            # ---- layer 2 -------------------------------------------------
            def epi2(t, po):
                ob = wpool.tile([128, OD], F32, tag="epi")
                nc.vector.tensor_tensor(out=ob[:], in0=po[:], in1=t_b2[:],
                                        op=mybir.AluOpType.add)
                nc.sync.dma_start(out=d_out[:, t, :], in_=ob[:])

            layer(u2full, t_u2, t_w2, OD, epi2, lambda b: None,
                  prefix=5)

    nc.compile()
    return nc


# ---------------------------------------------------------------- entry point
def kernel(x, W1, b1, W2, b2, edge_index):
    x = np.asarray(x, dtype=np.float32)
    W1 = np.asarray(W1, dtype=np.float32)
    b1 = np.asarray(b1, dtype=np.float32)
    W2 = np.asarray(W2, dtype=np.float32)
    b2 = np.asarray(b2, dtype=np.float32)
    edge_index = np.asarray(edge_index)

    ekey = hash(edge_index.tobytes())
    if ekey in _compiled_cache:
        nc, meta = _compiled_cache[ekey]
    else:
        meta = _prep_edges(edge_index)
        (runs, run_start, sec_base, sec_len, tot, idx_s, rel_s,
         tile_mms, sec_mm_len, mm_tot_pad, posmap) = meta
        nc = _build(runs, run_start, sec_base, sec_len, tot, tile_mms,
                    sec_mm_len, mm_tot_pad)
        _compiled_cache[ekey] = (nc, meta)
    (runs, run_start, sec_base, sec_len, tot, idx_s, rel_s,
     tile_mms, sec_mm_len, mm_tot_pad, posmap) = meta

    dst = np.asarray(edge_index[1], dtype=np.int64)
    deg_full = np.bincount(dst, minlength=N_NODES).astype(np.float32) + 1.0

    iota_np = np.tile(np.arange(TS, dtype=np.float16)[None, :], (128, 1))
    id32_np = np.eye(128, dtype=np.float32)
    id16_np = np.eye(128, dtype=np.float16)
    b1rep = np.tile(b1[None, :], (128, 1)).astype(np.float32)
    b2rep = np.tile(b2[None, :], (128, 1)).astype(np.float32)

    # host-precomputed u1 = dinv * x, padded per core, band-major table
    dinv_full = 1.0 / np.sqrt(deg_full)
    u1_pad = np.zeros((NC_CORES, SP, FD), dtype=np.float16)
    for k in range(NC_CORES):
        u1_pad[k, posmap[k * S:(k + 1) * S]] = (
            x[k * S:(k + 1) * S] * dinv_full[k * S:(k + 1) * S, None])
    u1full_host = np.zeros((SP * NC_CORES, 128), dtype=np.float16)
    for c in range(NC_CORES if False else len(CHB)):
        bt0, bt1 = BAND_T0[c] * TS, BAND_T1[c] * TS
        nrow = bt1 - bt0
        for k in range(NC_CORES):
            r0 = CHB[c] + k * nrow
            u1full_host[r0:r0 + nrow, 0:FD] = u1_pad[k, bt0:bt1]

    in_maps = []
    for k in range(NC_CORES):
        degs = np.ones((SP,), dtype=np.float32)
        degs[posmap[k * S:(k + 1) * S]] = deg_full[k * S:(k + 1) * S]
        in_maps.append({
            "u1full_in": u1full_host,
            "u1self": u1_pad[k].reshape(TPC, 128, FD).transpose(1, 0, 2).copy(),
            "deg_shard": degs.reshape(TPC, 128).T.copy(),
            "idx_stream": _wrap_idx(idx_s[k]),
            "rel_stream": _wrap_rel(rel_s[k]),
            "iota16": iota_np, "ident32": id32_np, "ident16": id16_np,
            "W1": W1, "b1rep": b1rep, "W2": W2, "b2rep": b2rep,
        })

    trace = bool(os.environ.get("BASS_TRACE"))
    res = run_bass_kernel_spmd(
        nc, in_maps, core_ids=list(range(NC_CORES)), trace=trace,
    )
    if trace and res.exec_time_ns is not None:
        print(f"HW exec time: {res.exec_time_ns} ns")
        kernel.last_exec_time_ns = res.exec_time_ns

    outs = []
    for k in range(NC_CORES):
        o = res.results[k]["out_shard"]          # [128, TPC, OD]
        flat = o.transpose(1, 0, 2).reshape(SP, OD)
        outs.append(flat[posmap[k * S:(k + 1) * S]])
    return np.concatenate(outs, axis=0)


# revision 6
# speedup vs baseline: 1.6784x; 1.0906x over previous
"""Bass/Trainium2 kernel for 2-layer GCN (nn_MeshGNN), 8 NeuronCores. v2.

Math (per layer, commuted form):
    A_hat = D^-1/2 (A+I) D^-1/2 ;  gcn(x) = A_hat x W + b
    u = dinv * x ;  agg[d] = sum_{e: dst=d} u[src[e]] + u[d]
    out = (dinv * agg) @ W + b           (layer 1 adds relu)

Distribution: nodes sharded by range across 8 cores (12500/core, padded to
12544 = 98 tiles of 128). The gather table u (fp16, 256B rows) is exchanged
in 4 row-BANDS via per-band AllGathers: band b holds tiles band_t0[b]:band_t1[b]
of EVERY core's shard, so chunk-c gathers unblock as soon as band c's
AllGather lands (layer 2's first AG fires mid-layer-1). The layer sweep is
chunk-major, accumulating per-tile partials in an SBUF f32 accumulator.
Edge streams (gather indices + relative-dst) are identical for both layers.
"""
import os
import numpy as np

import concourse.bacc as bacc
import concourse.mybir as mybir
from concourse.tile import TileContext
from concourse.bass_utils import run_bass_kernel_spmd

# ---------------------------------------------------------------- constants
N_NODES = 100000
NC_CORES = 8
S = 12500                 # nodes per core
TS = 128                  # dst-tile size
TPC = 98                  # dst tiles per core (98*128 = 12544)
SP = TPC * TS             # padded nodes per core
NCH = 4                   # src bands
BAND_TILES = [25, 25, 24, 24]
BAND_T0 = [sum(BAND_TILES[:i]) for i in range(NCH)]
BAND_T1 = [sum(BAND_TILES[:i + 1]) for i in range(NCH)]
CHR = [8 * 128 * bt for bt in BAND_TILES]   # table rows per band chunk
CHB = [sum(CHR[:i]) for i in range(NCH)]    # band chunk base row in big table
FD = 64                   # in/hidden feature dim
OD = 32                   # output dim
BLK = 1024                # gather block (slots per dma_gather, single-packet max)
OHG = 1024                # one-hot group (slots per DVE op)
F32 = mybir.dt.float32
F16 = mybir.dt.float16
I16 = mybir.dt.int16

_compiled_cache = {}


# ---------------------------------------------------------------- tile patch
def _install_tile_patch():
    """walrus here rejects >1 sync-wait on an InstDrain; split the Tile tail
    drain's waits across sequential drains (same engine => same semantics)."""
    from bass_rust import ScopedClock

    def _patched(self, tick_clock, wait_clock):
        drain_inst = self.nc.sync.drain()
        wait_clock.add_sem_waits(
            drain_inst.ins, ScopedClock({None: tick_clock.global_clock})
        )
        si = drain_inst.ins.sync_info
        waits = list(si.on_wait) if si and si.on_wait else []
        if len(waits) > 1:
            si.on_wait = waits[:1]
            for w in waits[1:]:
                extra = self.nc.sync.drain()
                extra.ins.sync_info = mybir.SyncInfo(on_wait=[w], on_update=[])
        self.nc.all_engine_barrier()
        assert self.sems is not None
        popped = self.nc._tile_sem_poison_stack.pop()
        assert popped is self._sem_poison
        self.nc.clear_and_free_semaphores(list(self.sems.allocated().values()))
        self.nc.all_engine_barrier()

    TileContext._drain_and_barrier = _patched


_install_tile_patch()


# ---------------------------------------------------------------- host prep
def _compute_pos_map(src, dst):
    """Within-core padded positions balancing per-(band,tile) in-degree.
    Nodes keep their band (so source-band profiles stay exact) but are dealt
    greedily among the band's tiles to flatten max-over-cores run lengths."""
    k_dst = dst // S
    pos0 = np.arange(N_NODES, dtype=np.int64) % S
    band_of_tile = np.zeros(TPC, dtype=np.int64)
    for b in range(NCH):
        band_of_tile[BAND_T0[b]:BAND_T1[b]] = b
    band_src = band_of_tile[(np.arange(N_NODES) % S) // TS]
    # per-node per-band in-degree profile
    prof = np.bincount(dst * NCH + band_src[src],
                       minlength=N_NODES * NCH).reshape(N_NODES, NCH)
    pos = np.zeros(N_NODES, dtype=np.int64)
    for k in range(NC_CORES):
        nodes_k = np.arange(k * S, (k + 1) * S)
        for b in range(NCH):
            t0, t1 = BAND_T0[b], BAND_T1[b]
            nt = t1 - t0
            sel = nodes_k[(pos0[nodes_k] // TS >= t0)
                          & (pos0[nodes_k] // TS < t1)]
            p = prof[sel].astype(np.float64)          # [n, NCH]
            order = np.argsort(-p.sum(axis=1))
            load = np.zeros((nt, NCH))
            cap = np.full(nt, TS, dtype=np.int64)
            cap[:] = TS
            slots_used = np.zeros(nt, dtype=np.int64)
            target = p.sum(axis=0) / nt
            for n in sel[order]:
                cand = np.where(slots_used < TS)[0]
                score = (load[cand] + prof[n][None, :] - target[None, :]
                         ).max(axis=1)
                j = cand[np.argmin(score)]
                load[j] += prof[n]
                pos[n] = (t0 + j) * TS + slots_used[j]
                slots_used[j] += 1
    return pos


def _prep_edges(edge_index):
    """Shared-run-structure edge streams, band chunks. Returns (runs_padded
    [NCH,TPC], run_start [NCH,TPC], sec_base [NCH], sec_len [NCH], tot,
    idx_streams [NC][tot], rel_streams [NC][tot])."""
    src = np.asarray(edge_index[0], dtype=np.int64)
    dst = np.asarray(edge_index[1], dtype=np.int64)
    posmap = _compute_pos_map(src, dst)

    # source table row within its band chunk
    k = src // S
    pos = posmap[src]
    stile = pos // TS
    band_of_tile = np.zeros(TPC, dtype=np.int64)
    for b in range(NCH):
        band_of_tile[BAND_T0[b]:BAND_T1[b]] = b
    band = band_of_tile[stile]
    t0 = np.asarray(BAND_T0, dtype=np.int64)
    trow = k * (np.asarray(BAND_TILES)[band] * TS) \
        + (stile - t0[band]) * TS + pos % TS

    core = dst // S
    dstloc = posmap[dst]
    tile = dstloc // TS

    key = (core * NCH + band) * TPC + tile
    counts = np.bincount(key, minlength=NC_CORES * NCH * TPC).reshape(
        NC_CORES, NCH, TPC
    )
    runs = counts.max(axis=0)                       # [NCH, TPC] exact max
    sec_len = runs.sum(axis=1)
    # pad each section to 128 so gather blocks stay 128-aligned in-section
    sec_len = ((sec_len + TS - 1) // TS) * TS
    sec_base = np.concatenate([[0], np.cumsum(sec_len)[:-1]])
    run_start = sec_base[:, None] + (np.cumsum(runs, axis=1) - runs)
    tot = int(sec_len.sum())

    # per-(c,t) matmul list: columns of 128 slots overlapping the run;
    # per-mm one-hot source = rel values masked to the run's slot range.
    tile_mms = [[None] * TPC for _ in range(NCH)]
    sec_mm_len = np.zeros(NCH, dtype=np.int64)
    mm_slot = []       # global mm -> (slot_lo, slot_hi, col_base)
    for c in range(NCH):
        m0 = 0
        for t in range(TPC):
            a = int(run_start[c, t]) - int(sec_base[c])
            b = a + int(runs[c, t])
            cols = range(a // TS, (b - 1) // TS + 1) if b > a else []
            tile_mms[c][t] = [(m0 + i, j) for i, j in enumerate(cols)]
            for j in cols:
                mm_slot.append((c, a, b, j * TS))
            m0 += len(tile_mms[c][t])
        sec_mm_len[c] = m0
    mm_tot = int(sec_mm_len.sum())
    mm_tot_pad = ((mm_tot + 7) // 8) * 8

    idx_streams, rel_streams = [], []
    for kk in range(NC_CORES):
        sel = core == kk
        c_k, t_k = band[sel], tile[sel]
        row_k = trow[sel]
        rel_k = (dstloc[sel] - t_k * TS).astype(np.float16)
        # within each (band, tile) run, order slots by ascending table row:
        # the gather's random reads become monotonic (DRAM-friendlier)
        order = np.lexsort((row_k, t_k, c_k))
        c_k, t_k, row_k, rel_k = c_k[order], t_k[order], row_k[order], rel_k[order]
        key_k = c_k * TPC + t_k
        cnt_k = np.bincount(key_k, minlength=NCH * TPC)
        grp_start = np.cumsum(cnt_k) - cnt_k
        within = np.arange(len(key_k)) - grp_start[key_k]
        slot = run_start.reshape(-1)[key_k] + within
        idx_s = np.zeros(tot, dtype=np.int16)
        rel_s = np.full(tot, -1.0, dtype=np.float16)
        idx_s[slot] = row_k.astype(np.int16)
        rel_s[slot] = rel_k
        # per-mm masked rel stream [mm_tot_pad * 128]
        relmm = np.full(mm_tot_pad * TS, -1.0, dtype=np.float16)
        for m, (c, a, b, col0) in enumerate(mm_slot):
            lo = max(a, col0)
            hi = min(b, col0 + TS)
            src = rel_s[int(sec_base[c]) + lo:int(sec_base[c]) + hi]
            relmm[m * TS + (lo - col0):m * TS + (hi - col0)] = src
        idx_streams.append(idx_s)
        rel_streams.append(relmm)
    return (runs, run_start, sec_base, sec_len, tot, idx_streams, rel_streams,
            tile_mms, sec_mm_len, mm_tot_pad, posmap)


def _wrap_idx(idx_s):
    """[tot] int16 -> [128, tot/16] wrapped + replicated across 8 groups."""
    tot = idx_s.shape[0]
    w = idx_s.reshape(tot // 16, 16).T              # [16, tot/16]
    return np.tile(w, (8, 1)).copy()                # [128, tot/16]


def _wrap_rel(rel_s):
    tot = rel_s.shape[0]
    return rel_s.reshape(tot // 128, 128).T.copy()  # [128, tot/128]


# ---------------------------------------------------------------- kernel build
def _build(runs, run_start, sec_base, sec_len, tot, tile_mms, sec_mm_len,
           mm_tot_pad):
    sec_mm_base = np.concatenate([[0], np.cumsum(sec_mm_len)[:-1]])
    nc = bacc.Bacc(None, target_bir_lowering=False, debug=False,
                   num_devices=NC_CORES, num_swdge_queues=4)

    # ---- I/O -------------------------------------------------------------
    d_u1f = nc.dram_tensor("u1full_in", [SP * NC_CORES, 128], F16,
                           kind="ExternalInput")
    d_u1s = nc.dram_tensor("u1self", [128, TPC, FD], F16, kind="ExternalInput")
    d_deg = nc.dram_tensor("deg_shard", [128, TPC], F32, kind="ExternalInput")
    d_idx = nc.dram_tensor("idx_stream", [128, tot // 16], I16, kind="ExternalInput")
    d_rel = nc.dram_tensor("rel_stream", [128, mm_tot_pad], F16, kind="ExternalInput")
    d_iota = nc.dram_tensor("iota16", [128, TS], F16, kind="ExternalInput")
    d_id32 = nc.dram_tensor("ident32", [128, 128], F32, kind="ExternalInput")
    d_id16 = nc.dram_tensor("ident16", [128, 128], F16, kind="ExternalInput")
    d_w1 = nc.dram_tensor("W1", [FD, FD], F32, kind="ExternalInput")
    d_b1 = nc.dram_tensor("b1rep", [128, FD], F32, kind="ExternalInput")
    d_w2 = nc.dram_tensor("W2", [FD, OD], F32, kind="ExternalInput")
    d_b2 = nc.dram_tensor("b2rep", [128, OD], F32, kind="ExternalInput")
    d_out = nc.dram_tensor("out_shard", [128, TPC, OD], F32, kind="ExternalOutput")

    cc_in2 = nc.dram_tensor("cc_in2", [SP, 128], F16, kind="Internal")
    u2big = nc.dram_tensor("u2full", [SP * NC_CORES, 128], F16,
                           kind="Internal", addr_space="Shared")
    u1full = [d_u1f[CHB[c]:CHB[c] + CHR[c], :] for c in range(NCH)]
    u2full = [u2big[CHB[c]:CHB[c] + CHR[c], :] for c in range(NCH)]

    with TileContext(nc) as tc:
        with (
            tc.tile_pool(name="const", bufs=1) as cpool,
            tc.tile_pool(name="stage", bufs=1) as spool,
            tc.tile_pool(name="msg", bufs=7) as mpool,
            tc.tile_pool(name="oh", bufs=3) as opool,
            tc.tile_pool(name="work", bufs=4) as wpool,
            tc.tile_pool(name="psA", bufs=4, space="PSUM") as psA,
            tc.tile_pool(name="psB", bufs=2, space="PSUM") as psB,
            tc.tile_pool(name="psC", bufs=2, space="PSUM") as psC,
        ):
            # ---- constants / streams ------------------------------------
            t_idx = cpool.tile([128, tot // 16], I16)
            nc.sync.dma_start(out=t_idx[:], in_=d_idx[:, :])
            t_rel = cpool.tile([128, mm_tot_pad], F16)
            nc.sync.dma_start(out=t_rel[:], in_=d_rel[:, :])
            t_iota = cpool.tile([128, TS], F16)
            nc.sync.dma_start(out=t_iota[:], in_=d_iota[:, :])
            t_id32 = cpool.tile([128, 128], F32)
            nc.sync.dma_start(out=t_id32[:], in_=d_id32[:, :])
            t_id16 = cpool.tile([128, 128], F16)
            nc.sync.dma_start(out=t_id16[:], in_=d_id16[:, :])
            t_w1 = cpool.tile([FD, FD], F32)
            nc.sync.dma_start(out=t_w1[:], in_=d_w1[:, :])
            t_b1 = cpool.tile([128, FD], F32)
            nc.sync.dma_start(out=t_b1[:], in_=d_b1[:, :])
            t_w2 = cpool.tile([FD, OD], F32)
            nc.sync.dma_start(out=t_w2[:], in_=d_w2[:, :])
            t_b2 = cpool.tile([128, OD], F32)
            nc.sync.dma_start(out=t_b2[:], in_=d_b2[:, :])

            # ---- dinv ----------------------------------------------------
            t_deg = cpool.tile([128, TPC], F32)
            nc.sync.dma_start(out=t_deg[:], in_=d_deg[:, :])
            t_dinv = cpool.tile([128, TPC], F32)
            nc.vector.reciprocal(out=t_dinv[:], in_=t_deg[:])
            nc.scalar.activation(out=t_dinv[:], in_=t_dinv[:],
                                 func=mybir.ActivationFunctionType.Sqrt)

            # ---- u1 table is host-precomputed (pure input function) ------
            t_u1 = spool.tile([128, TPC, FD], F16)
            nc.sync.dma_start(out=t_u1[:], in_=d_u1s[:, :, :])

            # ---- one shared layer (tile-major sweep, band AG overlap) ----
            t_u2 = spool.tile([128, TPC, FD], F16)

            def layer(ufull, u_stage, w_tile, outd, epilogue, post_band,
                      prefix=0):
                msg_tiles = {}
                oh_tiles = {}
                cursor_blk = [0] * NCH
                cursor_ohg = [0] * NCH

                def ensure(c, upto_slot, upto_mm):
                    while cursor_blk[c] * BLK < upto_slot:
                        bi = cursor_blk[c]
                        ln = min(BLK, int(sec_len[c]) - bi * BLK)
                        blk = mpool.tile([128, BLK // 128, 128], F16,
                                         tag=f"msg{c}")
                        a = int(sec_base[c]) + bi * BLK
                        nc.gpsimd.dma_gather(
                            blk[:, 0:ln // 128, :],
                            ufull[c],
                            t_idx[:, a // 16:(a + ln) // 16],
                            ln, ln, 128,
                            single_packet=True,
                            queue_num=c % 4,
                        )
                        msg_tiles[(c, bi)] = blk
                        cursor_blk[c] = bi + 1
                    while cursor_ohg[c] * 8 < upto_mm:
                        gi = cursor_ohg[c]
                        nb = min(8, int(sec_mm_len[c]) - gi * 8)
                        ohp = opool.tile([128, 8, TS], F16, tag=f"oh{c}")
                        g0 = int(sec_mm_base[c]) + gi * 8
                        nc.vector.tensor_tensor(
                            out=ohp[:, 0:nb, :],
                            in0=t_rel[:, g0:g0 + nb, None].to_broadcast(
                                [128, nb, TS]),
                            in1=t_iota[:, None, :].to_broadcast([128, nb, TS]),
                            op=mybir.AluOpType.is_equal,
                        )
                        oh_tiles[(c, gi)] = ohp
                        cursor_ohg[c] = gi + 1

                if prefix:
                    for c in range(NCH - 1):
                        ensure(c, prefix * BLK, prefix * 8)
                for t in range(TPC):
                    for c in range(NCH):
                        rs = int(run_start[c, t] - sec_base[c])
                        rl = int(runs[c, t])
                        if tile_mms[c][t]:
                            ensure(c, rs + rl, tile_mms[c][t][-1][0] + 1)
                    ps = psA.tile([128, FD], F32, tag="agg")
                    nc.tensor.matmul(out=ps[:], lhsT=t_id16[:],
                                     rhs=u_stage[:, t, :],
                                     start=True, stop=False)
                    mms = []
                    for c in range(NCH):
                        for m, j in tile_mms[c][t]:
                            mms.append((c, m, j))
                    for i, (c, m, j) in enumerate(mms):
                        oh = oh_tiles[(c, m // 8)]
                        mg = msg_tiles[(c, j * TS // BLK)]
                        nc.tensor.matmul(
                            out=ps[:],
                            lhsT=oh[:, m % 8, :],
                            rhs=mg[:, (j * TS % BLK) // 128, 0:FD],
                            start=False, stop=(i == len(mms) - 1),
                        )
                    assert mms
                    # epilogue: pre = dinv * agg; preT; po = preT.T @ W
                    pre = wpool.tile([128, FD], F32, tag="pre")
                    nc.vector.tensor_scalar(
                        out=pre[:], in0=ps[:], scalar1=t_dinv[:, t:t + 1],
                        scalar2=None, op0=mybir.AluOpType.mult,
                    )
                    pst = psB.tile([FD, 128], F32, tag="tr")
                    nc.tensor.transpose(out=pst[:], in_=pre[:],
                                        identity=t_id32[:])
                    preT = wpool.tile([FD, 128], F32, tag="preT")
                    nc.scalar.copy(out=preT[:], in_=pst[:])
                    po = psC.tile([128, outd], F32, tag="mm2")
                    nc.tensor.matmul(out=po[:], lhsT=preT[:], rhs=w_tile[:],
                                     start=True, stop=True)
                    epilogue(t, po)
                    for b in range(NCH):
                        if t == BAND_T1[b] - 1:
                            post_band(b)

            # ---- layer 1 -------------------------------------------------
            def epi1(t, po):
                xb = wpool.tile([128, FD], F32, tag="epi")
                nc.vector.tensor_tensor(out=xb[:], in0=po[:], in1=t_b1[:],
                                        op=mybir.AluOpType.add)
                nc.scalar.activation(
                    out=t_u2[:, t, :], in_=xb[:],
                    func=mybir.ActivationFunctionType.Relu,
                    scale=t_dinv[:, t:t + 1],
                )

            cc2_view = cc_in2.rearrange("(t p) f -> p t f", p=128)

            def post_band(b):
                nc.sync.dma_start(
                    out=cc2_view[:, BAND_T0[b]:BAND_T1[b], 0:FD],
                    in_=t_u2[:, BAND_T0[b]:BAND_T1[b], :],
                )
                nc.gpsimd.collective_compute(
                    "AllGather", mybir.AluOpType.bypass,
                    ins=[cc_in2[BAND_T0[b] * TS:BAND_T1[b] * TS, :]],
                    outs=[u2full[b]],
                    replica_groups=[list(range(NC_CORES))],
                )

            layer(u1full, t_u1, t_w1, FD, epi1, post_band, prefix=5)

            # ---- layer 2 -------------------------------------------------
            def epi2(t, po):
                ob = wpool.tile([128, OD], F32, tag="epi")
                nc.vector.tensor_tensor(out=ob[:], in0=po[:], in1=t_b2[:],
                                        op=mybir.AluOpType.add)
                nc.sync.dma_start(out=d_out[:, t, :], in_=ob[:])

            layer(u2full, t_u2, t_w2, OD, epi2, lambda b: None,
                  prefix=5)

    nc.compile()
    return nc


# ---------------------------------------------------------------- entry point
def kernel(x, W1, b1, W2, b2, edge_index):
    x = np.asarray(x, dtype=np.float32)
    W1 = np.asarray(W1, dtype=np.float32)
    b1 = np.asarray(b1, dtype=np.float32)
    W2 = np.asarray(W2, dtype=np.float32)
    b2 = np.asarray(b2, dtype=np.float32)
    edge_index = np.asarray(edge_index)

    ekey = hash(edge_index.tobytes())
    if ekey in _compiled_cache:
        nc, meta = _compiled_cache[ekey]
    else:
        meta = _prep_edges(edge_index)
        (runs, run_start, sec_base, sec_len, tot, idx_s, rel_s,
         tile_mms, sec_mm_len, mm_tot_pad, posmap) = meta
        nc = _build(runs, run_start, sec_base, sec_len, tot, tile_mms,
                    sec_mm_len, mm_tot_pad)
        _compiled_cache[ekey] = (nc, meta)
    (runs, run_start, sec_base, sec_len, tot, idx_s, rel_s,
     tile_mms, sec_mm_len, mm_tot_pad, posmap) = meta

    dst = np.asarray(edge_index[1], dtype=np.int64)
    deg_full = np.bincount(dst, minlength=N_NODES).astype(np.float32) + 1.0

    iota_np = np.tile(np.arange(TS, dtype=np.float16)[None, :], (128, 1))
    id32_np = np.eye(128, dtype=np.float32)
    id16_np = np.eye(128, dtype=np.float16)
    b1rep = np.tile(b1[None, :], (128, 1)).astype(np.float32)
    b2rep = np.tile(b2[None, :], (128, 1)).astype(np.float32)

    # host-precomputed u1 = dinv * x, padded per core, band-major table
    dinv_full = 1.0 / np.sqrt(deg_full)
    u1_pad = np.zeros((NC_CORES, SP, FD), dtype=np.float16)
    for k in range(NC_CORES):
        u1_pad[k, posmap[k * S:(k + 1) * S]] = (
            x[k * S:(k + 1) * S] * dinv_full[k * S:(k + 1) * S, None])
    u1full_host = np.zeros((SP * NC_CORES, 128), dtype=np.float16)
    for c in range(NC_CORES if False else len(CHB)):
        bt0, bt1 = BAND_T0[c] * TS, BAND_T1[c] * TS
        nrow = bt1 - bt0
        for k in range(NC_CORES):
            r0 = CHB[c] + k * nrow
            u1full_host[r0:r0 + nrow, 0:FD] = u1_pad[k, bt0:bt1]

    in_maps = []
    for k in range(NC_CORES):
        degs = np.ones((SP,), dtype=np.float32)
        degs[posmap[k * S:(k + 1) * S]] = deg_full[k * S:(k + 1) * S]
        in_maps.append({
            "u1full_in": u1full_host,
            "u1self": u1_pad[k].reshape(TPC, 128, FD).transpose(1, 0, 2).copy(),
            "deg_shard": degs.reshape(TPC, 128).T.copy(),
            "idx_stream": _wrap_idx(idx_s[k]),
            "rel_stream": _wrap_rel(rel_s[k]),
            "iota16": iota_np, "ident32": id32_np, "ident16": id16_np,
            "W1": W1, "b1rep": b1rep, "W2": W2, "b2rep": b2rep,
        })

    trace = bool(os.environ.get("BASS_TRACE"))
    res = run_bass_kernel_spmd(
        nc, in_maps, core_ids=list(range(NC_CORES)), trace=trace,
    )
    if trace and res.exec_time_ns is not None:
        print(f"HW exec time: {res.exec_time_ns} ns")
        kernel.last_exec_time_ns = res.exec_time_ns

    outs = []
    for k in range(NC_CORES):
        o = res.results[k]["out_shard"]          # [128, TPC, OD]
        flat = o.transpose(1, 0, 2).reshape(SP, OD)
        outs.append(flat[posmap[k * S:(k + 1) * S]])
    return np.concatenate(outs, axis=0)


# revision 7
# speedup vs baseline: 1.6889x; 1.0062x over previous
"""Bass/Trainium2 kernel for 2-layer GCN (nn_MeshGNN), 8 NeuronCores. v2.

Math (per layer, commuted form):
    A_hat = D^-1/2 (A+I) D^-1/2 ;  gcn(x) = A_hat x W + b
    u = dinv * x ;  agg[d] = sum_{e: dst=d} u[src[e]] + u[d]
    out = (dinv * agg) @ W + b           (layer 1 adds relu)

Distribution: nodes sharded by range across 8 cores (12500/core, padded to
12544 = 98 tiles of 128). The gather table u (fp16, 256B rows) is exchanged
in 4 row-BANDS via per-band AllGathers: band b holds tiles band_t0[b]:band_t1[b]
of EVERY core's shard, so chunk-c gathers unblock as soon as band c's
AllGather lands (layer 2's first AG fires mid-layer-1). The layer sweep is
chunk-major, accumulating per-tile partials in an SBUF f32 accumulator.
Edge streams (gather indices + relative-dst) are identical for both layers.
"""
import os
import numpy as np

import concourse.bacc as bacc
import concourse.mybir as mybir
from concourse.tile import TileContext
from concourse.bass_utils import run_bass_kernel_spmd

# ---------------------------------------------------------------- constants
N_NODES = 100000
NC_CORES = 8
S = 12500                 # nodes per core
TS = 128                  # dst-tile size
TPC = 98                  # dst tiles per core (98*128 = 12544)
SP = TPC * TS             # padded nodes per core
NCH = 4                   # src bands
BAND_TILES = [25, 25, 24, 24]
BAND_T0 = [sum(BAND_TILES[:i]) for i in range(NCH)]
BAND_T1 = [sum(BAND_TILES[:i + 1]) for i in range(NCH)]
CHR = [8 * 128 * bt for bt in BAND_TILES]   # table rows per band chunk
CHB = [sum(CHR[:i]) for i in range(NCH)]    # band chunk base row in big table
FD = 64                   # in/hidden feature dim
OD = 32                   # output dim
BLK = 1024                # gather block (slots per dma_gather, single-packet max)
OHG = 1024                # one-hot group (slots per DVE op)
F32 = mybir.dt.float32
F16 = mybir.dt.float16
I16 = mybir.dt.int16

_compiled_cache = {}


# ---------------------------------------------------------------- tile patch
def _install_tile_patch():
    """walrus here rejects >1 sync-wait on an InstDrain; split the Tile tail
    drain's waits across sequential drains (same engine => same semantics)."""
    from bass_rust import ScopedClock

    def _patched(self, tick_clock, wait_clock):
        drain_inst = self.nc.sync.drain()
        wait_clock.add_sem_waits(
            drain_inst.ins, ScopedClock({None: tick_clock.global_clock})
        )
        si = drain_inst.ins.sync_info
        waits = list(si.on_wait) if si and si.on_wait else []
        if len(waits) > 1:
            si.on_wait = waits[:1]
            for w in waits[1:]:
                extra = self.nc.sync.drain()
                extra.ins.sync_info = mybir.SyncInfo(on_wait=[w], on_update=[])
        self.nc.all_engine_barrier()
        assert self.sems is not None
        popped = self.nc._tile_sem_poison_stack.pop()
        assert popped is self._sem_poison
        self.nc.clear_and_free_semaphores(list(self.sems.allocated().values()))
        self.nc.all_engine_barrier()

    TileContext._drain_and_barrier = _patched


_install_tile_patch()


# ---------------------------------------------------------------- host prep
def _compute_pos_map(src, dst):
    """Within-core padded positions balancing per-(band,tile) in-degree.
    Nodes keep their band (so source-band profiles stay exact) but are dealt
    greedily among the band's tiles to flatten max-over-cores run lengths."""
    k_dst = dst // S
    pos0 = np.arange(N_NODES, dtype=np.int64) % S
    band_of_tile = np.zeros(TPC, dtype=np.int64)
    for b in range(NCH):
        band_of_tile[BAND_T0[b]:BAND_T1[b]] = b
    band_src = band_of_tile[(np.arange(N_NODES) % S) // TS]
    # per-node per-band in-degree profile
    prof = np.bincount(dst * NCH + band_src[src],
                       minlength=N_NODES * NCH).reshape(N_NODES, NCH)
    pos = np.zeros(N_NODES, dtype=np.int64)
    for k in range(NC_CORES):
        nodes_k = np.arange(k * S, (k + 1) * S)
        for b in range(NCH):
            t0, t1 = BAND_T0[b], BAND_T1[b]
            nt = t1 - t0
            sel = nodes_k[(pos0[nodes_k] // TS >= t0)
                          & (pos0[nodes_k] // TS < t1)]
            p = prof[sel].astype(np.float64)          # [n, NCH]
            order = np.argsort(-p.sum(axis=1))
            load = np.zeros((nt, NCH))
            cap = np.full(nt, TS, dtype=np.int64)
            cap[:] = TS
            slots_used = np.zeros(nt, dtype=np.int64)
            target = p.sum(axis=0) / nt
            for n in sel[order]:
                cand = np.where(slots_used < TS)[0]
                score = (load[cand] + prof[n][None, :] - target[None, :]
                         ).max(axis=1)
                j = cand[np.argmin(score)]
                load[j] += prof[n]
                pos[n] = (t0 + j) * TS + slots_used[j]
                slots_used[j] += 1
    return pos


def _prep_edges(edge_index):
    """Shared-run-structure edge streams, band chunks. Returns (runs_padded
    [NCH,TPC], run_start [NCH,TPC], sec_base [NCH], sec_len [NCH], tot,
    idx_streams [NC][tot], rel_streams [NC][tot])."""
    src = np.asarray(edge_index[0], dtype=np.int64)
    dst = np.asarray(edge_index[1], dtype=np.int64)
    posmap = _compute_pos_map(src, dst)

    # source table row within its band chunk
    k = src // S
    pos = posmap[src]
    stile = pos // TS
    band_of_tile = np.zeros(TPC, dtype=np.int64)
    for b in range(NCH):
        band_of_tile[BAND_T0[b]:BAND_T1[b]] = b
    band = band_of_tile[stile]
    t0 = np.asarray(BAND_T0, dtype=np.int64)
    trow = k * (np.asarray(BAND_TILES)[band] * TS) \
        + (stile - t0[band]) * TS + pos % TS

    core = dst // S
    dstloc = posmap[dst]
    tile = dstloc // TS

    key = (core * NCH + band) * TPC + tile
    counts = np.bincount(key, minlength=NC_CORES * NCH * TPC).reshape(
        NC_CORES, NCH, TPC
    )
    runs = counts.max(axis=0)                       # [NCH, TPC] exact max
    sec_len = runs.sum(axis=1)
    # pad each section to 128 so gather blocks stay 128-aligned in-section
    sec_len = ((sec_len + TS - 1) // TS) * TS
    sec_base = np.concatenate([[0], np.cumsum(sec_len)[:-1]])
    run_start = sec_base[:, None] + (np.cumsum(runs, axis=1) - runs)
    tot = int(sec_len.sum())

    # per-(c,t) matmul list: columns of 128 slots overlapping the run;
    # per-mm one-hot source = rel values masked to the run's slot range.
    tile_mms = [[None] * TPC for _ in range(NCH)]
    sec_mm_len = np.zeros(NCH, dtype=np.int64)
    mm_slot = []       # global mm -> (slot_lo, slot_hi, col_base)
    for c in range(NCH):
        m0 = 0
        for t in range(TPC):
            a = int(run_start[c, t]) - int(sec_base[c])
            b = a + int(runs[c, t])
            cols = range(a // TS, (b - 1) // TS + 1) if b > a else []
            tile_mms[c][t] = [(m0 + i, j) for i, j in enumerate(cols)]
            for j in cols:
                mm_slot.append((c, a, b, j * TS))
            m0 += len(tile_mms[c][t])
        sec_mm_len[c] = m0
    mm_tot = int(sec_mm_len.sum())
    mm_tot_pad = ((mm_tot + 7) // 8) * 8

    idx_streams, rel_streams = [], []
    for kk in range(NC_CORES):
        sel = core == kk
        c_k, t_k = band[sel], tile[sel]
        row_k = trow[sel]
        rel_k = (dstloc[sel] - t_k * TS).astype(np.float16)
        # within each (band, tile) run, order slots by ascending table row:
        # the gather's random reads become monotonic (DRAM-friendlier)
        order = np.lexsort((row_k, t_k, c_k))
        c_k, t_k, row_k, rel_k = c_k[order], t_k[order], row_k[order], rel_k[order]
        key_k = c_k * TPC + t_k
        cnt_k = np.bincount(key_k, minlength=NCH * TPC)
        grp_start = np.cumsum(cnt_k) - cnt_k
        within = np.arange(len(key_k)) - grp_start[key_k]
        slot = run_start.reshape(-1)[key_k] + within
        idx_s = np.zeros(tot, dtype=np.int16)
        rel_s = np.full(tot, -1.0, dtype=np.float16)
        idx_s[slot] = row_k.astype(np.int16)
        rel_s[slot] = rel_k
        # per-mm masked rel stream [mm_tot_pad * 128]
        relmm = np.full(mm_tot_pad * TS, -1.0, dtype=np.float16)
        for m, (c, a, b, col0) in enumerate(mm_slot):
            lo = max(a, col0)
            hi = min(b, col0 + TS)
            src = rel_s[int(sec_base[c]) + lo:int(sec_base[c]) + hi]
            relmm[m * TS + (lo - col0):m * TS + (hi - col0)] = src
        idx_streams.append(idx_s)
        rel_streams.append(relmm)
    return (runs, run_start, sec_base, sec_len, tot, idx_streams, rel_streams,
            tile_mms, sec_mm_len, mm_tot_pad, posmap)


def _wrap_idx(idx_s):
    """[tot] int16 -> [128, tot/16] wrapped + replicated across 8 groups."""
    tot = idx_s.shape[0]
    w = idx_s.reshape(tot // 16, 16).T              # [16, tot/16]
    return np.tile(w, (8, 1)).copy()                # [128, tot/16]


def _wrap_rel(rel_s):
    tot = rel_s.shape[0]
    return rel_s.reshape(tot // 128, 128).T.copy()  # [128, tot/128]


# ---------------------------------------------------------------- kernel build
def _build(runs, run_start, sec_base, sec_len, tot, tile_mms, sec_mm_len,
           mm_tot_pad):
    sec_mm_base = np.concatenate([[0], np.cumsum(sec_mm_len)[:-1]])
    nc = bacc.Bacc(None, target_bir_lowering=False, debug=False,
                   num_devices=NC_CORES, num_swdge_queues=4)

    # ---- I/O -------------------------------------------------------------
    d_u1f = nc.dram_tensor("u1full_in", [SP * NC_CORES, 128], F16,
                           kind="ExternalInput")
    d_u1s = nc.dram_tensor("u1self", [128, TPC, FD], F16, kind="ExternalInput")
    d_deg = nc.dram_tensor("deg_shard", [128, TPC], F32, kind="ExternalInput")
    d_idx = nc.dram_tensor("idx_stream", [128, tot // 16], I16, kind="ExternalInput")
    d_rel = nc.dram_tensor("rel_stream", [128, mm_tot_pad], F16, kind="ExternalInput")
    d_iota = nc.dram_tensor("iota16", [128, TS], F16, kind="ExternalInput")
    d_id32 = nc.dram_tensor("ident32", [128, 128], F32, kind="ExternalInput")
    d_id16 = nc.dram_tensor("ident16", [128, 128], F16, kind="ExternalInput")
    d_w1 = nc.dram_tensor("W1", [FD, FD], F32, kind="ExternalInput")
    d_b1 = nc.dram_tensor("b1rep", [128, FD], F32, kind="ExternalInput")
    d_w2 = nc.dram_tensor("W2", [FD, OD], F32, kind="ExternalInput")
    d_b2 = nc.dram_tensor("b2rep", [128, OD], F32, kind="ExternalInput")
    d_out = nc.dram_tensor("out_shard", [128, TPC, OD], F32, kind="ExternalOutput")

    cc_in2 = nc.dram_tensor("cc_in2", [SP, 128], F16, kind="Internal")
    u2big = nc.dram_tensor("u2full", [SP * NC_CORES, 128], F16,
                           kind="Internal", addr_space="Shared")
    u1full = [d_u1f[CHB[c]:CHB[c] + CHR[c], :] for c in range(NCH)]
    u2full = [u2big[CHB[c]:CHB[c] + CHR[c], :] for c in range(NCH)]

    with TileContext(nc) as tc:
        with (
            tc.tile_pool(name="const", bufs=1) as cpool,
            tc.tile_pool(name="stage", bufs=1) as spool,
            tc.tile_pool(name="msg", bufs=7) as mpool,
            tc.tile_pool(name="oh", bufs=3) as opool,
            tc.tile_pool(name="work", bufs=4) as wpool,
            tc.tile_pool(name="psA", bufs=4, space="PSUM") as psA,
            tc.tile_pool(name="psB", bufs=2, space="PSUM") as psB,
            tc.tile_pool(name="psC", bufs=2, space="PSUM") as psC,
        ):
            # ---- constants / streams ------------------------------------
            t_idx = cpool.tile([128, tot // 16], I16)
            nc.sync.dma_start(out=t_idx[:], in_=d_idx[:, :])
            t_rel = cpool.tile([128, mm_tot_pad], F16)
            nc.sync.dma_start(out=t_rel[:], in_=d_rel[:, :])
            t_iota = cpool.tile([128, TS], F16)
            nc.sync.dma_start(out=t_iota[:], in_=d_iota[:, :])
            t_id32 = cpool.tile([128, 128], F32)
            nc.sync.dma_start(out=t_id32[:], in_=d_id32[:, :])
            t_id16 = cpool.tile([128, 128], F16)
            nc.sync.dma_start(out=t_id16[:], in_=d_id16[:, :])
            t_w1 = cpool.tile([FD, FD], F32)
            nc.sync.dma_start(out=t_w1[:], in_=d_w1[:, :])
            t_b1 = cpool.tile([128, FD], F32)
            nc.sync.dma_start(out=t_b1[:], in_=d_b1[:, :])
            t_w2 = cpool.tile([FD, OD], F32)
            nc.sync.dma_start(out=t_w2[:], in_=d_w2[:, :])
            t_b2 = cpool.tile([128, OD], F32)
            nc.sync.dma_start(out=t_b2[:], in_=d_b2[:, :])

            # ---- dinv ----------------------------------------------------
            t_deg = cpool.tile([128, TPC], F32)
            nc.sync.dma_start(out=t_deg[:], in_=d_deg[:, :])
            t_dinv = cpool.tile([128, TPC], F32)
            nc.vector.reciprocal(out=t_dinv[:], in_=t_deg[:])
            nc.scalar.activation(out=t_dinv[:], in_=t_dinv[:],
                                 func=mybir.ActivationFunctionType.Sqrt)

            # ---- u1 table is host-precomputed (pure input function) ------
            t_u1 = spool.tile([128, TPC, FD], F16)
            nc.sync.dma_start(out=t_u1[:], in_=d_u1s[:, :, :])

            # ---- one shared layer (tile-major sweep, band AG overlap) ----
            t_u2 = spool.tile([128, TPC, FD], F16)

            def layer(ufull, u_stage, w_tile, outd, epilogue, post_band,
                      prefix=0):
                msg_tiles = {}
                oh_tiles = {}
                cursor_blk = [0] * NCH
                cursor_ohg = [0] * NCH
                qrot = [0]

                def ensure(c, upto_slot, upto_mm):
                    while cursor_blk[c] * BLK < upto_slot:
                        bi = cursor_blk[c]
                        ln = min(BLK, int(sec_len[c]) - bi * BLK)
                        blk = mpool.tile([128, BLK // 128, 128], F16,
                                         tag=f"msg{c}")
                        a = int(sec_base[c]) + bi * BLK
                        nc.gpsimd.dma_gather(
                            blk[:, 0:ln // 128, :],
                            ufull[c],
                            t_idx[:, a // 16:(a + ln) // 16],
                            ln, ln, 128,
                            single_packet=True,
                            queue_num=qrot[0] % 4,
                        )
                        qrot[0] += 1
                        msg_tiles[(c, bi)] = blk
                        cursor_blk[c] = bi + 1
                    while cursor_ohg[c] * 8 < upto_mm:
                        gi = cursor_ohg[c]
                        nb = min(8, int(sec_mm_len[c]) - gi * 8)
                        ohp = opool.tile([128, 8, TS], F16, tag=f"oh{c}")
                        g0 = int(sec_mm_base[c]) + gi * 8
                        nc.vector.tensor_tensor(
                            out=ohp[:, 0:nb, :],
                            in0=t_rel[:, g0:g0 + nb, None].to_broadcast(
                                [128, nb, TS]),
                            in1=t_iota[:, None, :].to_broadcast([128, nb, TS]),
                            op=mybir.AluOpType.is_equal,
                        )
                        oh_tiles[(c, gi)] = ohp
                        cursor_ohg[c] = gi + 1

                if prefix:
                    for c in range(NCH - 1):
                        ensure(c, prefix * BLK, prefix * 8)
                for t in range(TPC):
                    for c in range(NCH):
                        rs = int(run_start[c, t] - sec_base[c])
                        rl = int(runs[c, t])
                        if tile_mms[c][t]:
                            ensure(c, rs + rl, tile_mms[c][t][-1][0] + 1)
                    ps = psA.tile([128, FD], F32, tag="agg")
                    nc.tensor.matmul(out=ps[:], lhsT=t_id16[:],
                                     rhs=u_stage[:, t, :],
                                     start=True, stop=False)
                    mms = []
                    for c in range(NCH):
                        for m, j in tile_mms[c][t]:
                            mms.append((c, m, j))
                    for i, (c, m, j) in enumerate(mms):
                        oh = oh_tiles[(c, m // 8)]
                        mg = msg_tiles[(c, j * TS // BLK)]
                        nc.tensor.matmul(
                            out=ps[:],
                            lhsT=oh[:, m % 8, :],
                            rhs=mg[:, (j * TS % BLK) // 128, 0:FD],
                            start=False, stop=(i == len(mms) - 1),
                        )
                    assert mms
                    # epilogue: pre = dinv * agg; preT; po = preT.T @ W
                    pre = wpool.tile([128, FD], F32, tag="pre")
                    nc.vector.tensor_scalar(
                        out=pre[:], in0=ps[:], scalar1=t_dinv[:, t:t + 1],
                        scalar2=None, op0=mybir.AluOpType.mult,
                    )
                    pst = psB.tile([FD, 128], F32, tag="tr")
                    nc.tensor.transpose(out=pst[:], in_=pre[:],
                                        identity=t_id32[:])
                    preT = wpool.tile([FD, 128], F32, tag="preT")
                    nc.scalar.copy(out=preT[:], in_=pst[:])
                    po = psC.tile([128, outd], F32, tag="mm2")
                    nc.tensor.matmul(out=po[:], lhsT=preT[:], rhs=w_tile[:],
                                     start=True, stop=True)
                    epilogue(t, po)
                    for b in range(NCH):
                        if t == BAND_T1[b] - 1:
                            post_band(b)

            # ---- layer 1 -------------------------------------------------
            def epi1(t, po):
                xb = wpool.tile([128, FD], F32, tag="epi")
                nc.vector.tensor_tensor(out=xb[:], in0=po[:], in1=t_b1[:],
                                        op=mybir.AluOpType.add)
                nc.scalar.activation(
                    out=t_u2[:, t, :], in_=xb[:],
                    func=mybir.ActivationFunctionType.Relu,
                    scale=t_dinv[:, t:t + 1],
                )

            cc2_view = cc_in2.rearrange("(t p) f -> p t f", p=128)

            def post_band(b):
                nc.sync.dma_start(
                    out=cc2_view[:, BAND_T0[b]:BAND_T1[b], 0:FD],
                    in_=t_u2[:, BAND_T0[b]:BAND_T1[b], :],
                )
                nc.gpsimd.collective_compute(
                    "AllGather", mybir.AluOpType.bypass,
                    ins=[cc_in2[BAND_T0[b] * TS:BAND_T1[b] * TS, :]],
                    outs=[u2full[b]],
                    replica_groups=[list(range(NC_CORES))],
                )

            layer(u1full, t_u1, t_w1, FD, epi1, post_band, prefix=5)

            # ---- layer 2 -------------------------------------------------
            def epi2(t, po):
                ob = wpool.tile([128, OD], F32, tag="epi")
                nc.vector.tensor_tensor(out=ob[:], in0=po[:], in1=t_b2[:],
                                        op=mybir.AluOpType.add)
                nc.sync.dma_start(out=d_out[:, t, :], in_=ob[:])

            layer(u2full, t_u2, t_w2, OD, epi2, lambda b: None,
                  prefix=5)

    nc.compile()
    return nc


# ---------------------------------------------------------------- entry point
def kernel(x, W1, b1, W2, b2, edge_index):
    x = np.asarray(x, dtype=np.float32)
    W1 = np.asarray(W1, dtype=np.float32)
    b1 = np.asarray(b1, dtype=np.float32)
    W2 = np.asarray(W2, dtype=np.float32)
    b2 = np.asarray(b2, dtype=np.float32)
    edge_index = np.asarray(edge_index)

    ekey = hash(edge_index.tobytes())
    if ekey in _compiled_cache:
        nc, meta = _compiled_cache[ekey]
    else:
        meta = _prep_edges(edge_index)
        (runs, run_start, sec_base, sec_len, tot, idx_s, rel_s,
         tile_mms, sec_mm_len, mm_tot_pad, posmap) = meta
        nc = _build(runs, run_start, sec_base, sec_len, tot, tile_mms,
                    sec_mm_len, mm_tot_pad)
        _compiled_cache[ekey] = (nc, meta)
    (runs, run_start, sec_base, sec_len, tot, idx_s, rel_s,
     tile_mms, sec_mm_len, mm_tot_pad, posmap) = meta

    dst = np.asarray(edge_index[1], dtype=np.int64)
    deg_full = np.bincount(dst, minlength=N_NODES).astype(np.float32) + 1.0

    iota_np = np.tile(np.arange(TS, dtype=np.float16)[None, :], (128, 1))
    id32_np = np.eye(128, dtype=np.float32)
    id16_np = np.eye(128, dtype=np.float16)
    b1rep = np.tile(b1[None, :], (128, 1)).astype(np.float32)
    b2rep = np.tile(b2[None, :], (128, 1)).astype(np.float32)

    # host-precomputed u1 = dinv * x, padded per core, band-major table
    dinv_full = 1.0 / np.sqrt(deg_full)
    u1_pad = np.zeros((NC_CORES, SP, FD), dtype=np.float16)
    for k in range(NC_CORES):
        u1_pad[k, posmap[k * S:(k + 1) * S]] = (
            x[k * S:(k + 1) * S] * dinv_full[k * S:(k + 1) * S, None])
    u1full_host = np.zeros((SP * NC_CORES, 128), dtype=np.float16)
    for c in range(NC_CORES if False else len(CHB)):
        bt0, bt1 = BAND_T0[c] * TS, BAND_T1[c] * TS
        nrow = bt1 - bt0
        for k in range(NC_CORES):
            r0 = CHB[c] + k * nrow
            u1full_host[r0:r0 + nrow, 0:FD] = u1_pad[k, bt0:bt1]

    in_maps = []
    for k in range(NC_CORES):
        degs = np.ones((SP,), dtype=np.float32)
        degs[posmap[k * S:(k + 1) * S]] = deg_full[k * S:(k + 1) * S]
        in_maps.append({
            "u1full_in": u1full_host,
            "u1self": u1_pad[k].reshape(TPC, 128, FD).transpose(1, 0, 2).copy(),
            "deg_shard": degs.reshape(TPC, 128).T.copy(),
            "idx_stream": _wrap_idx(idx_s[k]),
            "rel_stream": _wrap_rel(rel_s[k]),
            "iota16": iota_np, "ident32": id32_np, "ident16": id16_np,
            "W1": W1, "b1rep": b1rep, "W2": W2, "b2rep": b2rep,
        })

    trace = bool(os.environ.get("BASS_TRACE"))
    res = run_bass_kernel_spmd(
        nc, in_maps, core_ids=list(range(NC_CORES)), trace=trace,
    )
    if trace and res.exec_time_ns is not None:
        print(f"HW exec time: {res.exec_time_ns} ns")
        kernel.last_exec_time_ns = res.exec_time_ns

    outs = []
    for k in range(NC_CORES):
        o = res.results[k]["out_shard"]          # [128, TPC, OD]
        flat = o.transpose(1, 0, 2).reshape(SP, OD)
        outs.append(flat[posmap[k * S:(k + 1) * S]])
    return np.concatenate(outs, axis=0)
